# revision 8
# baseline (speedup 1.0000x reference)
"""Trainium2 Bass kernel for a 2-layer GAT (cross-attention fusion + 8-head GAT
+ 1-head GAT) distributed over 8 NeuronCores.

Strategy (src-sharded message passing, all gathers local):
  - Phase A: per-node feature transforms sharded by src node (NS/8 per core),
    activations kept transposed [feat, node] so matmuls contract on partitions.
    Each core writes a local gather-table row per owned node:
      [z(512) | s_src(8) | s_dst(8) | pad] bf16, 1280B rows.
  - A small AllGather replicates a padded per-node s_dst table (256B rows).
  - Layer-1 aggregation: each core processes the edges whose src it owns.
    dma_gather fetches z rows from the LOCAL table (int16 idx ok), a second
    gather fetches s_dst by global dst. Per-edge exp(leakyrelu(s_src+s_dst))
    weights are applied, and one-hot matmuls accumulate exp-weighted partial
    sums + denominators per global dst tile in PSUM.
  - One bf16 ReduceScatter sums partials [ND0, 520] across cores; each core
    ends with its own dst range = its layer-2 src shard.
  - Normalize + ELU + z2 projection per owned node -> local layer-2 table,
    then the same aggregation scheme for layer 2 and a final ReduceScatter.
"""
import os
import sys
import math

sys.path.insert(0, "/opt/trn_rl_repo")

import numpy as np
import ml_dtypes

import concourse.bass as bass
import concourse.bacc as bacc
import concourse.tile as tile
import concourse.mybir as mybir
from concourse.bass_utils import run_bass_kernel_spmd

BF16 = ml_dtypes.bfloat16
NCORE = 8
F = 512          # fused dim
H = 8            # layer-1 heads
OUT = 128        # layer-2 out dim
ROW1 = 640       # layer-1 table row, bf16 elems (1280B): [z 512|s_src 8|s_dst 8|pad]
SROW = 128       # s_dst table row, bf16 elems (256B)
ROW2 = 256       # layer-2 table row (512B): [z2 128|s2src 1|pad]
PR1 = 520        # layer-1 partial row: [h 512 | den 8]
PR2 = 129        # layer-2 partial row: [h 128 | den 1]
BUDGET_CHUNKS = 4    # chunks per dma_gather call (4 SWDGE queues in flight)


def _ceil(a, b):
    return -(-a // b)


def _wrap_idx(idx):
    """[S] -> [128, S//16] int16, wrapped in 16 partitions, replicated 8x."""
    w = idx.reshape(-1, 16).T.astype(np.int16)
    return np.ascontiguousarray(np.tile(w, (8, 1)))


def _sched(src, dst, shard, n_dst):
    """Static per-dst-tile schedule, uniform across cores.

    Returns (chunkmeta, calls, S, g_idx[8,S], sd_idx[8,S], dcol[8,S])."""
    T = _ceil(n_dst, 128)
    owner = src // shard
    percore = []
    cnt = np.zeros((NCORE, T), np.int64)
    for c in range(NCORE):
        m = owner == c
        s_loc = (src[m] - c * shard).astype(np.int64)
        d = dst[m].astype(np.int64)
        o = np.argsort(d, kind="stable")
        s_loc, d = s_loc[o], d[o]
        t = d // 128
        cnt[c] = np.bincount(t, minlength=T)
        percore.append((s_loc, d, t))
    C = np.maximum(1, _ceil_arr(cnt.max(axis=0), 128))
    chunkmeta = []
    tile_off = np.zeros(T + 1, np.int64)
    for t in range(T):
        for p in range(int(C[t])):
            chunkmeta.append((t, p == 0, p == C[t] - 1))
        tile_off[t + 1] = tile_off[t] + C[t] * 128
    S = int(tile_off[-1])
    calls = []
    cur_off = cur_n = 0
    for t in range(T):
        if cur_n + C[t] > BUDGET_CHUNKS and cur_n > 0:
            calls.append((cur_off, cur_n))
            cur_off += cur_n
            cur_n = 0
        cur_n += int(C[t])
    calls.append((cur_off, cur_n))
    g = np.zeros((NCORE, S), np.int64)
    sd = np.zeros((NCORE, S), np.int64)
    dcol = np.full((NCORE, S), -1.0, np.float32)
    for c in range(NCORE):
        s_loc, d, t = percore[c]
        starts = np.searchsorted(t, np.arange(T))
        pos = np.arange(len(t)) - starts[t]
        slot = tile_off[t] + pos
        g[c, slot] = s_loc
        sd[c, slot] = d
        dcol[c, slot] = (d - t * 128).astype(np.float32)
    return chunkmeta, calls, S, g, sd, dcol


def _ceil_arr(a, b):
    return -(-a // b)


def _prep(inputs):
    img = np.asarray(inputs["image_features"], np.float32)
    blk = np.asarray(inputs["block_features"], np.float32)
    W_img = np.asarray(inputs["W_img"], np.float32)
    W_blk = np.asarray(inputs["W_blk"], np.float32)
    Wv = np.asarray(inputs["Wv"], np.float32)
    bv = np.asarray(inputs["bv"], np.float32)
    We = np.asarray(inputs["We"], np.float32)
    be = np.asarray(inputs["be"], np.float32)
    fc1 = np.asarray(inputs["fc1"], np.float32)
    attn1 = np.asarray(inputs["attn1"], np.float32)
    fc2 = np.asarray(inputs["fc2"], np.float32)
    attn2 = np.asarray(inputs["attn2"], np.float32)
    e0s = np.asarray(inputs["edge0_src"], np.int64)
    e0d = np.asarray(inputs["edge0_dst"], np.int64)
    e1s = np.asarray(inputs["edge1_src"], np.int64)
    e1d = np.asarray(inputs["edge1_dst"], np.int64)
    ND0 = int(inputs["n_dst0"])
    ND1 = int(inputs["n_dst1"])

    NS, IMG = img.shape
    BLK = blk.shape[1]
    assert W_img.shape == (F, IMG) and W_blk.shape == (F, BLK)
    assert fc1.shape == (H, F // H, F) and fc2.shape[1] == OUT
    assert NS % NCORE == 0 and ND0 % NCORE == 0 and ND1 % NCORE == 0
    assert IMG % 128 == 0 and BLK % 128 == 0
    SS, DS0, DS1 = NS // NCORE, ND0 // NCORE, ND1 // NCORE

    O = F // H
    # host-derived weights
    wimgT = np.ascontiguousarray(W_img.T).astype(BF16)          # [IMG, F]
    wblkT = np.ascontiguousarray(W_blk.T).astype(BF16)          # [BLK, F]
    wv = Wv.astype(BF16)                                        # [F, F] lhsT
    we = We.astype(BF16)
    fc1T = np.ascontiguousarray(fc1.reshape(F, F).T).astype(BF16)   # [F, (h o)]
    a_src = np.einsum("hof,ho->fh", fc1, attn1[:, :O])
    a_dst = np.einsum("hof,ho->fh", fc1, attn1[:, O:])
    acat = np.concatenate([a_src, a_dst], axis=1).astype(BF16)  # [F, 16]
    fc2T = np.ascontiguousarray(fc2[0].T).astype(BF16)          # [F, OUT]
    a2s = np.tile(attn2[0, :OUT].astype(np.float32), (128, 1))  # [128, OUT]
    a2d = np.tile(attn2[0, OUT:].astype(np.float32), (128, 1))
    MB = F // 128
    biasv = np.ascontiguousarray(bv.reshape(MB, 128).T).astype(np.float32)
    biase = np.ascontiguousarray(be.reshape(MB, 128).T).astype(np.float32)
    iota = np.tile(np.arange(128, dtype=np.float32), (128, 1))
    ident = np.eye(128, dtype=np.float32).astype(BF16)

    cm1, calls1, S1, g1, sd1, dc1 = _sched(e0s, e0d, SS, ND0)
    cm2, calls2, S2, g2, sd2, dc2 = _sched(e1s, e1d, DS0, ND1)

    shared = dict(wimgT=wimgT, wblkT=wblkT, wv=wv, we=we, fc1T=fc1T, acat=acat,
                  fc2T=fc2T, a2s=a2s, a2d=a2d, biasv=biasv, biase=biase,
                  iota=iota, ident=ident,
                  tick=np.zeros((128, 1), np.float32))
    in_maps = []
    for c in range(NCORE):
        m = dict(shared)
        m["imgT"] = np.ascontiguousarray(
            img[c * SS:(c + 1) * SS].T).astype(BF16)
        m["blkT"] = np.ascontiguousarray(
            blk[c * SS:(c + 1) * SS].T).astype(BF16)
        m["g1"] = _wrap_idx(g1[c])
        m["sd1"] = _wrap_idx(sd1[c])
        m["dc1"] = np.ascontiguousarray(dc1[c].reshape(-1, 128).T)
        m["g2"] = _wrap_idx(g2[c])
        m["sd2"] = _wrap_idx(sd2[c])
        m["dc2"] = np.ascontiguousarray(dc2[c].reshape(-1, 128).T)
        in_maps.append(m)

    cfg = dict(NS=NS, IMG=IMG, BLK=BLK, ND0=ND0, ND1=ND1,
               SS=SS, DS0=DS0, DS1=DS1,
               cm1=tuple(cm1), calls1=tuple(calls1), S1=S1,
               cm2=tuple(cm2), calls2=tuple(calls2), S2=S2)
    return cfg, in_maps


# ---------------------------------------------------------------- device code

AGG_LEVEL = int(os.environ.get("GAT_AGG_LEVEL", "3"))
P4_LEVEL = int(os.environ.get("GAT_P4_LEVEL", "2"))


def _agg_layer(nc, tc, ctx, cfg, *, table, sdt, gsb, sdsb, dcsb, iota_sb,
               calls, cm, row, srow, prow, zw, nh, partials, n_dst, rep=0):
    """Shared edge-aggregation pipeline for both GAT layers.

    row: gather row width (elems); srow: s_dst row width; prow: partial row;
    zw: z width (512 or 128); nh: heads (8 or 1).
    Layer-1 rows: [z 512 | s_src 8 | s_dst 8 | pad]; s-gather gives s_dst.
    Layer-2 rows: [z2 128 | s2src 1 | pad]; s-gather gives s2dst.
    """
    bf16 = mybir.dt.bfloat16
    f32 = mybir.dt.float32
    maxch = max(n for _, n in calls)
    gb = ctx.enter_context(tc.tile_pool(name=f"gb{zw}_{rep}", bufs=6))
    sdb = ctx.enter_context(tc.tile_pool(name=f"sdb{zw}_{rep}", bufs=6))
    ohb = ctx.enter_context(tc.tile_pool(name=f"ohb{zw}_{rep}", bufs=2))
    zsb = ctx.enter_context(tc.tile_pool(name=f"zsb{zw}_{rep}", bufs=2))
    escb = ctx.enter_context(tc.tile_pool(name=f"escb{zw}_{rep}", bufs=2))
    stg = ctx.enter_context(tc.tile_pool(name=f"stg{zw}_{rep}", bufs=3))
    ph = ctx.enter_context(tc.tile_pool(name=f"ph{zw}_{rep}", bufs=2, space="PSUM"))
    pd = ctx.enter_context(tc.tile_pool(name=f"pd{zw}_{rep}", bufs=2, space="PSUM"))

    zcol = zw + nh  # matmul rhs width: [z | exp]
    Exp = mybir.ActivationFunctionType.Exp
    cur_ph = cur_pd = None
    for ci, (coff, nch) in enumerate(calls):
        soff = coff * 128
        nidx = nch * 128
        gt = gb.tile([128, nch * row], bf16, tag="gt")
        nc.gpsimd.dma_gather(
            gt[:].rearrange("p (c e) -> p c e", e=row),
            table[:, :], gsb[:, soff // 16:(soff + nidx) // 16],
            nidx, nidx, row, queue_num=ci % 4)
        sdt_t = sdb.tile([128, nch * srow], bf16, tag="sdt")
        nc.gpsimd.dma_gather(
            sdt_t[:].rearrange("p (c e) -> p c e", e=srow),
            sdt[:, :], sdsb[:, soff // 16:(soff + nidx) // 16],
            nidx, nidx, srow, queue_num=(ci + 2) % 4)
        g3 = gt[:].rearrange("p (c e) -> p c e", e=row)
        s3 = sdt_t[:].rearrange("p (c e) -> p c e", e=srow)
        if AGG_LEVEL <= 1:
            st = stg.tile([128, prow], bf16, tag="st")
            nc.vector.tensor_copy(st[:, 0:prow], gt[:, 0:prow])
            nc.vector.tensor_copy(st[:, 0:srow // 2], sdt_t[:, 0:srow // 2])
            nc.sync.dma_start(partials[0:128, 0:prow], st[:, 0:prow])
            continue
        # one-hot [128e, nch, 128d]
        oh = ohb.tile([128, nch * 128], bf16, tag="oh")
        nc.vector.tensor_tensor(
            oh[:].rearrange("p (c d) -> p c d", d=128),
            iota_sb[:].unsqueeze(1).broadcast_to([128, nch, 128]),
            dcsb[:, coff:coff + nch].unsqueeze(2).broadcast_to([128, nch, 128]),
            mybir.AluOpType.is_equal)
        # escore = s_src (gathered row) + s_dst (s-gather)
        esc = escb.tile([128, nch * nh], f32, tag="esc")
        e3 = esc[:].rearrange("p (c h) -> p c h", h=nh)
        nc.vector.tensor_tensor(e3, g3[:, :, zw:zw + nh], s3[:, :, 0:nh],
                                mybir.AluOpType.add)
        nc.vector.scalar_tensor_tensor(esc[:], esc[:], 0.01, esc[:],
                                       mybir.AluOpType.mult,
                                       mybir.AluOpType.max)
        zs = zsb.tile([128, nch * zcol], bf16, tag="zs")
        z3 = zs[:].rearrange("p (c e) -> p c e", e=zcol)
        nc.scalar.activation(z3[:, :, zw:zcol], e3, Exp)
        # z * exp broadcast per head
        nc.vector.tensor_tensor(
            z3[:, :, 0:zw].rearrange("p c (h o) -> p c h o", h=nh),
            g3[:, :, 0:zw].rearrange("p c (h o) -> p c h o", h=nh),
            z3[:, :, zw:zcol].unsqueeze(3).broadcast_to(
                [128, nch, nh, zw // nh]),
            mybir.AluOpType.mult)
        if zw + nh <= 512:
            # single fused matmul per chunk: rhs [z | exp]
            for j in range(nch):
                t, first, last = cm[coff + j]
                if first:
                    cur_ph = ph.tile([128, zw + nh], f32, tag="ph")
                nc.tensor.matmul(cur_ph[:], oh[:, j * 128:(j + 1) * 128],
                                 zs[:, j * zcol:(j + 1) * zcol],
                                 start=first, stop=last)
                if last:
                    st = stg.tile([128, prow], bf16, tag="st")
                    nc.vector.tensor_copy(st[:, 0:zw + nh], cur_ph[:])
                    rows = min(128, n_dst - t * 128)
                    nc.sync.dma_start(
                        partials[t * 128:t * 128 + rows, 0:zw + nh],
                        st[:rows, 0:zw + nh])
        else:
            # two contiguous accumulation passes per tile (h then den):
            # interleaving open PSUM groups on PE crashes HW.
            jt = 0
            while jt < nch:
                t = cm[coff + jt][0]
                span = 0
                while jt + span < nch and cm[coff + jt + span][0] == t:
                    span += 1
                cur_ph = ph.tile([128, zw], f32, tag="ph")
                cur_pd = pd.tile([128, nh], f32, tag="pd")
                for q in range(span):
                    j = jt + q
                    nc.tensor.matmul(cur_ph[:], oh[:, j * 128:(j + 1) * 128],
                                     zs[:, j * zcol:j * zcol + zw],
                                     start=(q == 0), stop=(q == span - 1))
                for q in range(span):
                    j = jt + q
                    nc.tensor.matmul(cur_pd[:], oh[:, j * 128:(j + 1) * 128],
                                     zs[:, j * zcol + zw:(j + 1) * zcol],
                                     start=(q == 0), stop=(q == span - 1))
                st = stg.tile([128, prow], bf16, tag="st")
                nc.vector.tensor_copy(st[:, 0:zw], cur_ph[:])
                nc.vector.tensor_copy(st[:, zw:zw + nh], cur_pd[:])
                rows = min(128, n_dst - t * 128)
                nc.sync.dma_start(
                    partials[t * 128:t * 128 + rows, 0:zw + nh],
                    st[:rows, 0:zw + nh])
                jt += span


STOP_STAGE = int(os.environ.get("GAT_STOP_STAGE", "9"))



def _build(cfg):
    stop = STOP_STAGE
    REPEAT = cfg.get("repeat", 1)
    bf16 = mybir.dt.bfloat16
    f32 = mybir.dt.float32
    i16 = mybir.dt.int16
    NS, IMG, BLK = cfg["NS"], cfg["IMG"], cfg["BLK"]
    ND0, ND1 = cfg["ND0"], cfg["ND1"]
    SS, DS0, DS1 = cfg["SS"], cfg["DS0"], cfg["DS1"]
    S1, S2 = cfg["S1"], cfg["S2"]
    KI, KB, MB = IMG // 128, BLK // 128, F // 128
    O = F // H

    nc = bacc.Bacc("TRN2", target_bir_lowering=False, debug=False,
                   enable_asserts=True, num_devices=NCORE,
                   num_swdge_queues=4)

    def param(name, shape, dt):
        return nc.declare_dram_parameter(name, list(shape), dt, isOutput=False)

    imgT = param("imgT", [IMG, SS], bf16)
    blkT = param("blkT", [BLK, SS], bf16)
    wimgT = param("wimgT", [IMG, F], bf16)
    wblkT = param("wblkT", [BLK, F], bf16)
    wv = param("wv", [F, F], bf16)
    we = param("we", [F, F], bf16)
    fc1T = param("fc1T", [F, F], bf16)
    acat = param("acat", [F, 16], bf16)
    fc2T = param("fc2T", [F, OUT], bf16)
    a2s = param("a2s", [128, OUT], f32)
    a2d = param("a2d", [128, OUT], f32)
    biasv = param("biasv", [128, MB], f32)
    biase = param("biase", [128, MB], f32)
    iota = param("iota", [128, 128], f32)
    ident = param("ident", [128, 128], bf16)
    g1 = param("g1", [128, S1 // 16], i16)
    sd1 = param("sd1", [128, S1 // 16], i16)
    dc1 = param("dc1", [128, S1 // 128], f32)
    g2 = param("g2", [128, S2 // 16], i16)
    sd2 = param("sd2", [128, S2 // 16], i16)
    dc2 = param("dc2", [128, S2 // 128], f32)
    tick = param("tick", [128, 1], f32)
    out = nc.declare_dram_parameter("out", [DS1, OUT], f32, isOutput=True)
    tock = nc.declare_dram_parameter("tock", [128, 1], f32, isOutput=True)

    table1 = nc.dram_tensor("table1", [SS, ROW1], bf16)
    sdsh1 = nc.dram_tensor("sdsh1", [SS, SROW], bf16)
    sdt1 = nc.dram_tensor("sdt1", [NS, SROW], bf16, addr_space="Shared")
    sdt1b = nc.dram_tensor("sdt1b", [NS, SROW], bf16)
    partials1 = nc.dram_tensor("partials1", [ND0, PR1], bf16)
    rs1 = nc.dram_tensor("rs1", [DS0, PR1], bf16)
    table2 = nc.dram_tensor("table2", [DS0, ROW2], bf16)
    sdsh2 = nc.dram_tensor("sdsh2", [DS0, SROW], bf16)
    sdt2 = nc.dram_tensor("sdt2", [ND0, SROW], bf16, addr_space="Shared")
    sdt2b = nc.dram_tensor("sdt2b", [ND0, SROW], bf16)
    partials2 = nc.dram_tensor("partials2", [ND1, PR2], bf16)
    rs2 = nc.dram_tensor("rs2", [DS1, PR2], bf16)

    Sig = mybir.ActivationFunctionType.Sigmoid
    Exp = mybir.ActivationFunctionType.Exp
    TT = nc.vector.tensor_tensor
    MUL = mybir.AluOpType.mult
    ADD = mybir.AluOpType.add

    from contextlib import ExitStack
    if True:
      with tile.TileContext(nc) as tc, ExitStack() as top:
        res = top.enter_context(tc.tile_pool(name="res", bufs=1))
        # resident weights / constants
        wimg_sb = res.tile([128, KI * F], bf16)
        nc.sync.dma_start(wimg_sb[:].rearrange("p (k m) -> p k m", k=KI),
                          wimgT[:, :].rearrange("(k p) m -> p k m", p=128))
        wblk_sb = res.tile([128, KB * F], bf16)
        nc.sync.dma_start(wblk_sb[:].rearrange("p (k m) -> p k m", k=KB),
                          wblkT[:, :].rearrange("(k p) m -> p k m", p=128))
        wv_sb = res.tile([128, MB * F], bf16)
        nc.sync.dma_start(wv_sb[:].rearrange("p (k m) -> p k m", k=MB),
                          wv[:, :].rearrange("(k p) m -> p k m", p=128))
        we_sb = res.tile([128, MB * F], bf16)
        nc.sync.dma_start(we_sb[:].rearrange("p (k m) -> p k m", k=MB),
                          we[:, :].rearrange("(k p) m -> p k m", p=128))
        fc1_sb = res.tile([128, MB * F], bf16)
        nc.sync.dma_start(fc1_sb[:].rearrange("p (k m) -> p k m", k=MB),
                          fc1T[:, :].rearrange("(k p) m -> p k m", p=128))
        acat_sb = res.tile([128, MB * 16], bf16)
        nc.sync.dma_start(acat_sb[:].rearrange("p (k m) -> p k m", k=MB),
                          acat[:, :].rearrange("(k p) m -> p k m", p=128))
        fc2_sb = res.tile([128, MB * OUT], bf16)
        nc.sync.dma_start(fc2_sb[:].rearrange("p (k m) -> p k m", k=MB),
                          fc2T[:, :].rearrange("(k p) m -> p k m", p=128))
        a2s_sb = res.tile([128, OUT], f32)
        nc.sync.dma_start(a2s_sb[:], a2s[:, :])
        a2d_sb = res.tile([128, OUT], f32)
        nc.sync.dma_start(a2d_sb[:], a2d[:, :])
        bv_sb = res.tile([128, MB], f32)
        nc.sync.dma_start(bv_sb[:], biasv[:, :])
        be_sb = res.tile([128, MB], f32)
        nc.sync.dma_start(be_sb[:], biase[:, :])
        iota_sb = res.tile([128, 128], f32)
        nc.sync.dma_start(iota_sb[:], iota[:, :])
        id_sb = res.tile([128, 128], bf16)
        nc.sync.dma_start(id_sb[:], ident[:, :])
        g1_sb = res.tile([128, S1 // 16], i16)
        nc.sync.dma_start(g1_sb[:], g1[:, :])
        sd1_sb = res.tile([128, S1 // 16], i16)
        nc.sync.dma_start(sd1_sb[:], sd1[:, :])
        dc1_sb = res.tile([128, S1 // 128], f32)
        nc.sync.dma_start(dc1_sb[:], dc1[:, :])
        g2_sb = res.tile([128, S2 // 16], i16)
        nc.sync.dma_start(g2_sb[:], g2[:, :])
        sd2_sb = res.tile([128, S2 // 16], i16)
        nc.sync.dma_start(sd2_sb[:], sd2[:, :])
        dc2_sb = res.tile([128, S2 // 128], f32)
        nc.sync.dma_start(dc2_sb[:], dc2[:, :])

        # chain for timing
        tk = res.tile([128, 1], f32)
        nc.sync.dma_start(tk[:], tick[:, :])
        nc.sync.dma_start(tock[:, :], tk[:])
        if stop < 9:
            zo = res.tile([128, OUT], f32)
            nc.vector.memset(zo[:], 0.0)
            for tt in range(_ceil(DS1, 128)):
                rows = min(128, DS1 - tt * 128)
                nc.sync.dma_start(out[tt * 128:tt * 128 + rows, :],
                                  zo[:rows, :])

        for _rep in range(REPEAT):
          if _rep:
              # serialize repeats so repeat-K timing measures a full
              # dependency-honest iteration (idempotent reps would
              # otherwise overlap through untracked DRAM reuse)
              tc.strict_bb_all_engine_barrier()
          # ---------------- Phase A ----------------
          WA = min(500, SS)
          with ExitStack() as pa:
              rhsp = pa.enter_context(tc.tile_pool(name=f"parhs{_rep}", bufs=2))
              sbp = pa.enter_context(tc.tile_pool(name=f"pasb{_rep}", bufs=2))
              psp = pa.enter_context(tc.tile_pool(name=f"paps{_rep}", bufs=4, space="PSUM"))
              pst = pa.enter_context(tc.tile_pool(name=f"patr{_rep}", bufs=2, space="PSUM"))
              stp = pa.enter_context(tc.tile_pool(name=f"past{_rep}", bufs=3))
              for nt in range(_ceil(SS, WA)):
                  n0 = nt * WA
                  w = min(WA, SS - n0)
                  x_sb = rhsp.tile([128, KI * w], bf16, tag="x")
                  nc.sync.dma_start(
                      x_sb[:].rearrange("p (k n) -> p k n", k=KI),
                      imgT[:, n0:n0 + w].rearrange("(k p) n -> p k n", p=128))
                  b_sb = rhsp.tile([128, KB * w], bf16, tag="b")
                  nc.sync.dma_start(
                      b_sb[:].rearrange("p (k n) -> p k n", k=KB),
                      blkT[:, n0:n0 + w].rearrange("(k p) n -> p k n", p=128))

                  def mm(lhs_sb, rhs_sb, K, m, width):
                      ps = psp.tile([128, width], f32, tag="ps")
                      for k in range(K):
                          nc.tensor.matmul(
                              ps[:],
                              lhs_sb[:, (k * F + m * 128):(k * F + m * 128) + 128],
                              rhs_sb[:, k * width:(k + 1) * width],
                              start=(k == 0), stop=(k == K - 1))
                      return ps

                  fi_sb = sbp.tile([128, MB * w], bf16, tag="fi")
                  ti_sb = sbp.tile([128, MB * w], bf16, tag="ti")
                  av_sb = sbp.tile([128, MB * w], bf16, tag="av")
                  ae_sb = sbp.tile([128, MB * w], bf16, tag="ae")
                  for m in range(MB):
                      ps = mm(wimg_sb, x_sb, KI, m, w)
                      nc.vector.tensor_copy(fi_sb[:, m * w:(m + 1) * w], ps[:])
                  for m in range(MB):
                      ps = mm(wblk_sb, b_sb, KB, m, w)
                      nc.vector.tensor_copy(ti_sb[:, m * w:(m + 1) * w], ps[:])
                  for m in range(MB):
                      ps = mm(wv_sb, fi_sb, MB, m, w)
                      nc.scalar.activation(av_sb[:, m * w:(m + 1) * w], ps[:],
                                           Sig, bias=bv_sb[:, m:m + 1])
                  for m in range(MB):
                      ps = mm(we_sb, ti_sb, MB, m, w)
                      nc.scalar.activation(ae_sb[:, m * w:(m + 1) * w], ps[:],
                                           Sig, bias=be_sb[:, m:m + 1])
                  fu_sb = sbp.tile([128, MB * w], bf16, tag="fu")
                  TT(fu_sb[:], av_sb[:], fi_sb[:], MUL)
                  TT(ae_sb[:], ae_sb[:], ti_sb[:], MUL)
                  TT(fu_sb[:], fu_sb[:], ae_sb[:], ADD)
                  z_sb = sbp.tile([128, MB * w], bf16, tag="z")
                  for m in range(MB):
                      ps = mm(fc1_sb, fu_sb, MB, m, w)
                      nc.vector.tensor_copy(z_sb[:, m * w:(m + 1) * w], ps[:])
                  pss = psp.tile([128, w], f32, tag="ps")
                  for k in range(MB):
                      nc.tensor.matmul(pss[:16, :], acat_sb[:, k * 16:(k + 1) * 16],
                                       fu_sb[:, k * w:(k + 1) * w],
                                       start=(k == 0), stop=(k == MB - 1))
                  s_sb = sbp.tile([16, w], bf16, tag="s")
                  nc.vector.tensor_copy(s_sb[:], pss[:16, :])
                  for b0 in range(0, w, 128):
                      wb = min(128, w - b0)
                      st = stp.tile([128, ROW1], bf16, tag="t1")
                      nc.vector.memset(st[:, F + 16:ROW1], 0.0)
                      for m in range(MB):
                          ptr = pst.tile([128, 128], bf16, tag="tr")
                          nc.tensor.matmul(ptr[:wb, :],
                                           z_sb[:, m * w + b0:m * w + b0 + wb],
                                           id_sb[:], is_transpose=True)
                          nc.vector.tensor_copy(
                              st[:wb, m * 128:(m + 1) * 128], ptr[:wb, :])
                      ptr = pst.tile([128, 128], bf16, tag="tr")
                      nc.tensor.matmul(ptr[:wb, :16], s_sb[:, b0:b0 + wb],
                                       id_sb[:16, :16], is_transpose=True)
                      nc.vector.tensor_copy(st[:wb, F:F + 16], ptr[:wb, :16])
                      nc.sync.dma_start(table1[n0 + b0:n0 + b0 + wb, :],
                                        st[:wb, :])
                      st2 = stp.tile([128, SROW], bf16, tag="t2")
                      nc.vector.memset(st2[:, 8:SROW], 0.0)
                      nc.vector.tensor_copy(st2[:wb, 0:8], st[:wb, F + 8:F + 16])
                      nc.sync.dma_start(sdsh1[n0 + b0:n0 + b0 + wb, :],
                                        st2[:wb, :])

          if stop >= 2:
              nc.gpsimd.collective_compute(
                  "AllGather", mybir.AluOpType.bypass,
                  replica_groups=[list(range(NCORE))],
                  ins=[sdsh1[:, :]], outs=[sdt1[:, :]])
              nc.sync.dma_start(sdt1b[:, :], sdt1[:, :])

          # ---------------- Layer-1 aggregation ----------------
          with ExitStack() as ag1:
            if stop >= 3:
              _agg_layer(nc, tc, ag1, cfg, table=table1, sdt=sdt1b,
                         gsb=g1_sb, sdsb=sd1_sb, dcsb=dc1_sb, iota_sb=iota_sb,
                         calls=cfg["calls1"], cm=cfg["cm1"],
                         row=ROW1, srow=SROW, prow=PR1, zw=F, nh=H,
                         partials=partials1, n_dst=ND0, rep=_rep)

          if stop >= 4:
              nc.gpsimd.collective_compute(
                  "ReduceScatter", ADD, replica_groups=[list(range(NCORE))],
                  ins=[partials1[:, :]], outs=[rs1[:, :]])

          # ---------------- normalize + layer-2 tables ----------------
          with ExitStack() as p4:
            if stop >= 5:
              sbp = p4.enter_context(tc.tile_pool(name=f"n2sb{_rep}", bufs=3))
              psp = p4.enter_context(tc.tile_pool(name=f"n2ps{_rep}", bufs=2, space="PSUM"))
              ptp = p4.enter_context(tc.tile_pool(name=f"n2pt{_rep}", bufs=2, space="PSUM"))
              for tt in range(_ceil(DS0, 128)):
                  r0 = tt * 128
                  rows = min(128, DS0 - r0)
                  hs = sbp.tile([128, PR1], bf16, tag="hs")
                  nc.sync.dma_start(hs[:rows, :], rs1[r0:r0 + rows, :])
                  rden = sbp.tile([128, H], f32, tag="rd")
                  nc.vector.reciprocal(rden[:rows, :], hs[:rows, F:F + H])
                  hraw = sbp.tile([128, F], f32, tag="hraw")
                  TT(hraw[:rows, :].rearrange("p (h o) -> p h o", h=H),
                     hs[:rows, 0:F].rearrange("p (h o) -> p h o", h=H),
                     rden[:rows, :].unsqueeze(2).broadcast_to(
                         [rows, H, F // H]),
                     MUL)
                  t1 = sbp.tile([128, F], f32, tag="t1")
                  nc.vector.tensor_scalar_min(t1[:rows, :], hraw[:rows, :], 0.0)
                  nc.scalar.activation(t1[:rows, :], t1[:rows, :], Exp)
                  h1 = sbp.tile([128, F], bf16, tag="h1")
                  nc.vector.scalar_tensor_tensor(
                      h1[:rows, :], t1[:rows, :], -1.0, hraw[:rows, :],
                      ADD, mybir.AluOpType.max)
                  h1t = sbp.tile([128, MB * 128], bf16, tag="h1t")
                  for m in range(MB):
                      ptr = ptp.tile([128, 128], bf16, tag="tr")
                      nc.tensor.matmul(ptr[:, :rows],
                                       h1[:rows, m * 128:(m + 1) * 128],
                                       id_sb[:rows, :rows], is_transpose=True)
                      nc.vector.tensor_copy(h1t[:, m * 128:m * 128 + rows],
                                            ptr[:, :rows])
                  pz2 = psp.tile([128, OUT], f32, tag="z2")
                  for k in range(MB):
                      nc.tensor.matmul(pz2[:rows, :],
                                       h1t[:, k * 128:k * 128 + rows],
                                       fc2_sb[:, k * OUT:(k + 1) * OUT],
                                       start=(k == 0), stop=(k == MB - 1))
                  scr = sbp.tile([128, OUT], f32, tag="scr")
                  s2s = sbp.tile([128, 1], f32, tag="s2s")
                  s2d = sbp.tile([128, 1], f32, tag="s2d")
                  if P4_LEVEL >= 3:
                      nc.vector.tensor_tensor_reduce(
                          scr[:rows, :], pz2[:rows, :], a2s_sb[:rows, :], 1.0,
                          0.0, MUL, ADD, s2s[:rows, :])
                      nc.vector.tensor_tensor_reduce(
                          scr[:rows, :], pz2[:rows, :], a2d_sb[:rows, :], 1.0,
                          0.0, MUL, ADD, s2d[:rows, :])
                  else:
                      nc.vector.tensor_tensor(scr[:rows, :], pz2[:rows, :],
                                              a2s_sb[:rows, :], MUL)
                      nc.vector.reduce_sum(s2s[:rows, :], scr[:rows, :],
                                           mybir.AxisListType.X)
                      nc.vector.tensor_tensor(scr[:rows, :], pz2[:rows, :],
                                              a2d_sb[:rows, :], MUL)
                      nc.vector.reduce_sum(s2d[:rows, :], scr[:rows, :],
                                           mybir.AxisListType.X)
                  st = sbp.tile([128, ROW2], bf16, tag="st")
                  nc.vector.memset(st[:, OUT + 1:ROW2], 0.0)
                  nc.vector.tensor_copy(st[:rows, 0:OUT], pz2[:rows, :])
                  nc.vector.tensor_copy(st[:rows, OUT:OUT + 1], s2s[:rows, :])
                  nc.sync.dma_start(table2[r0:r0 + rows, :], st[:rows, :])
                  st2 = sbp.tile([128, SROW], bf16, tag="st2")
                  nc.vector.memset(st2[:, 1:SROW], 0.0)
                  nc.vector.tensor_copy(st2[:rows, 0:1], s2d[:rows, :])
                  nc.sync.dma_start(sdsh2[r0:r0 + rows, :], st2[:rows, :])

          if stop >= 6:
              nc.gpsimd.collective_compute(
                  "AllGather", mybir.AluOpType.bypass,
                  replica_groups=[list(range(NCORE))],
                  ins=[sdsh2[:, :]], outs=[sdt2[:, :]])
              nc.sync.dma_start(sdt2b[:, :], sdt2[:, :])

          # ---------------- Layer-2 aggregation ----------------
          with ExitStack() as ag2:
            if stop >= 7:
              _agg_layer(nc, tc, ag2, cfg, table=table2, sdt=sdt2b,
                         gsb=g2_sb, sdsb=sd2_sb, dcsb=dc2_sb, iota_sb=iota_sb,
                         calls=cfg["calls2"], cm=cfg["cm2"],
                         row=ROW2, srow=SROW, prow=PR2, zw=OUT, nh=1,
                         partials=partials2, n_dst=ND1, rep=_rep)

          if stop >= 8:
              nc.gpsimd.collective_compute(
                  "ReduceScatter", ADD, replica_groups=[list(range(NCORE))],
                  ins=[partials2[:, :]], outs=[rs2[:, :]])

          # ---------------- final normalize ----------------
          with ExitStack() as p8:
            if stop >= 9:
              sbp = p8.enter_context(tc.tile_pool(name=f"fsb{_rep}", bufs=3))
              for tt in range(_ceil(DS1, 128)):
                  r0 = tt * 128
                  rows = min(128, DS1 - r0)
                  hs = sbp.tile([128, PR2], bf16, tag="hs")
                  nc.sync.dma_start(hs[:rows, :], rs2[r0:r0 + rows, :])
                  rden = sbp.tile([128, 1], f32, tag="rd")
                  nc.vector.reciprocal(rden[:rows, :], hs[:rows, OUT:OUT + 1])
                  ot = sbp.tile([128, OUT], f32, tag="ot")
                  TT(ot[:rows, :], hs[:rows, 0:OUT],
                     rden[:rows, :].broadcast_to([rows, OUT]), MUL)
                  nc.sync.dma_start(out[r0:r0 + rows, :], ot[:rows, :])

    nc.compile()
    return nc


_CACHE = {}


def _get_nc(cfg):
    key = repr(sorted((k, v) for k, v in cfg.items()))
    if key not in _CACHE:
        _CACHE[key] = _build(cfg)
    return _CACHE[key]


def kernel(**inputs) -> np.ndarray:
    cfg, in_maps = _prep(inputs)
    nc = _get_nc(cfg)
    res = run_bass_kernel_spmd(nc, in_maps, core_ids=list(range(NCORE)))
    return np.concatenate([res.results[c]["out"] for c in range(NCORE)],
                          axis=0)



# revision 14
# speedup vs baseline: 1.0488x; 1.0488x over previous
"""Trainium2 Bass kernel for a 2-layer GAT (cross-attention fusion + 8-head GAT
+ 1-head GAT) distributed over 8 NeuronCores.

Strategy (src-sharded message passing, all gathers local):
  - Phase A: per-node feature transforms sharded by src node (NS/8 per core),
    activations kept transposed [feat, node] so matmuls contract on partitions.
    Each core writes a local gather-table row per owned node:
      [z(512) | s_src(8) | s_dst(8) | pad] bf16, 1280B rows.
  - A small AllGather replicates a padded per-node s_dst table (256B rows).
  - Layer-1 aggregation: each core processes the edges whose src it owns.
    dma_gather fetches z rows from the LOCAL table (int16 idx ok), a second
    gather fetches s_dst by global dst. Per-edge exp(leakyrelu(s_src+s_dst))
    weights are applied, and one-hot matmuls accumulate exp-weighted partial
    sums + denominators per global dst tile in PSUM.
  - One bf16 ReduceScatter sums partials [ND0, 520] across cores; each core
    ends with its own dst range = its layer-2 src shard.
  - Normalize + ELU + z2 projection per owned node -> local layer-2 table,
    then the same aggregation scheme for layer 2 and a final ReduceScatter.
"""
import os
import sys
import math

sys.path.insert(0, "/opt/trn_rl_repo")

import numpy as np
import ml_dtypes

import concourse.bass as bass
import concourse.bacc as bacc
import concourse.tile as tile
import concourse.mybir as mybir
from concourse.bass_utils import run_bass_kernel_spmd

BF16 = ml_dtypes.bfloat16
NCORE = 8
F = 512          # fused dim
H = 8            # layer-1 heads
OUT = 128        # layer-2 out dim
ROW1 = 640       # layer-1 table row, bf16 elems (1280B): [z 512|s_src 8|s_dst 8|pad]
SROW = 128       # s_dst table row, bf16 elems (256B)
ROW2 = 256       # layer-2 table row (512B): [z2 128|s2src 1|pad]
PR1 = 520        # layer-1 partial row: [h 512 | den 8]
PR2 = 129        # layer-2 partial row: [h 128 | den 1]
BUDGET_CHUNKS = 4    # chunks per dma_gather call (4 SWDGE queues in flight)


def _ceil(a, b):
    return -(-a // b)


def _wrap_idx(idx):
    """[S] -> [128, S//16] int16, wrapped in 16 partitions, replicated 8x."""
    w = idx.reshape(-1, 16).T.astype(np.int16)
    return np.ascontiguousarray(np.tile(w, (8, 1)))


def _routing(t, n_dst, own, half):
    """Store routing for dst tile t: list of (sel, src_lo, ln, dst_lo).

    Global partial rows are re-laid-out as two half tensors so a
    ReduceScatter over each delivers core c exactly the first/second half
    of its owned range: half-0 rows = concat_c [own*c, own*c+half),
    half-1 rows = concat_c [own*c+half, own*(c+1))."""
    r0, r1 = 128 * t, min(128 * (t + 1), n_dst)
    out = []
    r = r0
    while r < r1:
        c, loc = r // own, r % own
        sel = 0 if loc < half else 1
        seg_end = own * c + (half if sel == 0 else own)
        ln = min(r1, seg_end) - r
        dst = half * c + (loc if sel == 0 else loc - half)
        out.append((sel, r - r0, ln, dst))
        r += ln
    return tuple(out)


def _sched(src, dst, shard, n_dst, own):
    """Static per-dst-tile schedule, uniform across cores.

    Tiles are processed group-A-first (tiles containing any first-half
    row of a core's owned range) so the half-0 partials tensor is
    complete mid-aggregation and its ReduceScatter overlaps the rest.

    Returns (chunkmeta, calls, S, g[8,S], sd[8,S], dcol[8,S],
             routing, split_call, half)."""
    T = _ceil(n_dst, 128)
    half = own // 2
    owner = src // shard
    percore = []
    cnt = np.zeros((NCORE, T), np.int64)
    for c in range(NCORE):
        m = owner == c
        s_loc = (src[m] - c * shard).astype(np.int64)
        d = dst[m].astype(np.int64)
        o = np.argsort(d, kind="stable")
        s_loc, d = s_loc[o], d[o]
        t = d // 128
        cnt[c] = np.bincount(t, minlength=T)
        percore.append((s_loc, d, t))
    C = np.maximum(1, _ceil_arr(cnt.max(axis=0), 128))
    routing = {t: _routing(t, n_dst, own, half) for t in range(T)}
    grpA = [t for t in range(T) if any(s == 0 for s, _, _, _ in routing[t])]
    grpB = [t for t in range(T) if t not in set(grpA)]
    perm = grpA + grpB
    chunkmeta = []
    tile_base = np.zeros(T, np.int64)
    off = 0
    nchunks_A = 0
    for t in perm:
        tile_base[t] = off
        for p in range(int(C[t])):
            chunkmeta.append((t, p == 0, p == C[t] - 1))
        off += C[t] * 128
        if t in set(grpA):
            nchunks_A += int(C[t])
    S = int(off)
    calls = []
    cur_off = cur_n = 0
    for t in perm:
        if cur_n + C[t] > BUDGET_CHUNKS and cur_n > 0:
            calls.append((cur_off, cur_n))
            cur_off += cur_n
            cur_n = 0
        cur_n += int(C[t])
    calls.append((cur_off, cur_n))
    # call whose chunks complete group A -> hook point for the first RS
    split_call = 0
    acc = 0
    for i, (coff, n) in enumerate(calls):
        acc += n
        if acc >= nchunks_A:
            split_call = i
            break
    g = np.zeros((NCORE, S), np.int64)
    sd = np.zeros((NCORE, S), np.int64)
    dcol = np.full((NCORE, S), -1.0, np.float32)
    for c in range(NCORE):
        s_loc, d, t = percore[c]
        starts = np.searchsorted(t, np.arange(T))
        pos = np.arange(len(t)) - starts[t]
        slot = tile_base[t] + pos
        g[c, slot] = s_loc
        sd[c, slot] = d
        dcol[c, slot] = (d - t * 128).astype(np.float32)
    routing_t = tuple(routing[t] for t in range(T))
    return chunkmeta, calls, S, g, sd, dcol, routing_t, split_call, half


def _ceil_arr(a, b):
    return -(-a // b)


def _prep(inputs):
    img = np.asarray(inputs["image_features"], np.float32)
    blk = np.asarray(inputs["block_features"], np.float32)
    W_img = np.asarray(inputs["W_img"], np.float32)
    W_blk = np.asarray(inputs["W_blk"], np.float32)
    Wv = np.asarray(inputs["Wv"], np.float32)
    bv = np.asarray(inputs["bv"], np.float32)
    We = np.asarray(inputs["We"], np.float32)
    be = np.asarray(inputs["be"], np.float32)
    fc1 = np.asarray(inputs["fc1"], np.float32)
    attn1 = np.asarray(inputs["attn1"], np.float32)
    fc2 = np.asarray(inputs["fc2"], np.float32)
    attn2 = np.asarray(inputs["attn2"], np.float32)
    e0s = np.asarray(inputs["edge0_src"], np.int64)
    e0d = np.asarray(inputs["edge0_dst"], np.int64)
    e1s = np.asarray(inputs["edge1_src"], np.int64)
    e1d = np.asarray(inputs["edge1_dst"], np.int64)
    ND0 = int(inputs["n_dst0"])
    ND1 = int(inputs["n_dst1"])

    NS, IMG = img.shape
    BLK = blk.shape[1]
    assert W_img.shape == (F, IMG) and W_blk.shape == (F, BLK)
    assert fc1.shape == (H, F // H, F) and fc2.shape[1] == OUT
    assert NS % NCORE == 0 and ND0 % NCORE == 0 and ND1 % NCORE == 0
    assert IMG % 128 == 0 and BLK % 128 == 0
    SS, DS0, DS1 = NS // NCORE, ND0 // NCORE, ND1 // NCORE

    O = F // H
    # host-derived weights
    wimgT = np.ascontiguousarray(W_img.T).astype(BF16)          # [IMG, F]
    wblkT = np.ascontiguousarray(W_blk.T).astype(BF16)          # [BLK, F]
    wv = Wv.astype(BF16)                                        # [F, F] lhsT
    we = We.astype(BF16)
    fc1T = np.ascontiguousarray(fc1.reshape(F, F).T).astype(BF16)   # [F, (h o)]
    a_src = np.einsum("hof,ho->fh", fc1, attn1[:, :O])
    a_dst = np.einsum("hof,ho->fh", fc1, attn1[:, O:])
    acat = np.concatenate([a_src, a_dst], axis=1).astype(BF16)  # [F, 16]
    fc2T = np.ascontiguousarray(fc2[0].T).astype(BF16)          # [F, OUT]
    a2s = np.tile(attn2[0, :OUT].astype(np.float32), (128, 1))  # [128, OUT]
    a2d = np.tile(attn2[0, OUT:].astype(np.float32), (128, 1))
    MB = F // 128
    biasv = np.ascontiguousarray(bv.reshape(MB, 128).T).astype(np.float32)
    biase = np.ascontiguousarray(be.reshape(MB, 128).T).astype(np.float32)
    iota = np.tile(np.arange(128, dtype=np.float32), (128, 1))
    ident = np.eye(128, dtype=np.float32).astype(BF16)

    (cm1, calls1, S1, g1, sd1, dc1,
     rt1, sp1, half1) = _sched(e0s, e0d, SS, ND0, ND0 // NCORE)
    (cm2, calls2, S2, g2, sd2, dc2,
     rt2, sp2, half2) = _sched(e1s, e1d, DS0, ND1, ND1 // NCORE)

    shared = dict(wimgT=wimgT, wblkT=wblkT, wv=wv, we=we, fc1T=fc1T, acat=acat,
                  fc2T=fc2T, a2s=a2s, a2d=a2d, biasv=biasv, biase=biase,
                  iota=iota, ident=ident,
                  tick=np.zeros((128, 1), np.float32))
    in_maps = []
    for c in range(NCORE):
        m = dict(shared)
        m["imgT"] = np.ascontiguousarray(
            img[c * SS:(c + 1) * SS].T).astype(BF16)
        m["blkT"] = np.ascontiguousarray(
            blk[c * SS:(c + 1) * SS].T).astype(BF16)
        m["g1"] = _wrap_idx(g1[c])
        m["sd1"] = _wrap_idx(sd1[c])
        m["dc1"] = np.ascontiguousarray(dc1[c].reshape(-1, 128).T)
        m["g2"] = _wrap_idx(g2[c])
        m["sd2"] = _wrap_idx(sd2[c])
        m["dc2"] = np.ascontiguousarray(dc2[c].reshape(-1, 128).T)
        in_maps.append(m)

    cfg = dict(NS=NS, IMG=IMG, BLK=BLK, ND0=ND0, ND1=ND1,
               SS=SS, DS0=DS0, DS1=DS1,
               cm1=tuple(cm1), calls1=tuple(calls1), S1=S1,
               cm2=tuple(cm2), calls2=tuple(calls2), S2=S2,
               rt1=rt1, sp1=sp1, half1=half1,
               rt2=rt2, sp2=sp2, half2=half2)
    return cfg, in_maps


# ---------------------------------------------------------------- device code

AGG_LEVEL = int(os.environ.get("GAT_AGG_LEVEL", "3"))
P4_LEVEL = int(os.environ.get("GAT_P4_LEVEL", "2"))


def _agg_layer(nc, tc, ctx, cfg, *, table, sdt, gsb, sdsb, dcsb, iota_sb,
               calls, cm, row, srow, prow, zw, nh, pa, pb, routing,
               hooks=None, rep=0):
    """Shared edge-aggregation pipeline for both GAT layers.

    row: gather row width (elems); srow: s_dst row width; prow: partial row;
    zw: z width (512 or 128); nh: heads (8 or 1).
    Layer-1 rows: [z 512 | s_src 8 | s_dst 8 | pad]; s-gather gives s_dst.
    Layer-2 rows: [z2 128 | s2src 1 | pad]; s-gather gives s2dst.
    """
    bf16 = mybir.dt.bfloat16
    f32 = mybir.dt.float32
    maxch = max(n for _, n in calls)
    gb = ctx.enter_context(tc.tile_pool(name=f"gb{zw}_{rep}", bufs=6))
    sdb = ctx.enter_context(tc.tile_pool(name=f"sdb{zw}_{rep}", bufs=6))
    ohb = ctx.enter_context(tc.tile_pool(name=f"ohb{zw}_{rep}", bufs=2))
    zsb = ctx.enter_context(tc.tile_pool(name=f"zsb{zw}_{rep}", bufs=2))
    escb = ctx.enter_context(tc.tile_pool(name=f"escb{zw}_{rep}", bufs=2))
    stg = ctx.enter_context(tc.tile_pool(name=f"stg{zw}_{rep}", bufs=3))
    ph = ctx.enter_context(tc.tile_pool(name=f"ph{zw}_{rep}", bufs=2, space="PSUM"))
    pd = ctx.enter_context(tc.tile_pool(name=f"pd{zw}_{rep}", bufs=2, space="PSUM"))

    zcol = zw + nh  # matmul rhs width: [z | exp]
    Exp = mybir.ActivationFunctionType.Exp
    cur_ph = cur_pd = None
    for ci, (coff, nch) in enumerate(calls):
        soff = coff * 128
        nidx = nch * 128
        gt = gb.tile([128, nch * row], bf16, tag="gt")
        nc.gpsimd.dma_gather(
            gt[:].rearrange("p (c e) -> p c e", e=row),
            table[:, :], gsb[:, soff // 16:(soff + nidx) // 16],
            nidx, nidx, row, queue_num=ci % 4)
        sdt_t = sdb.tile([128, nch * srow], bf16, tag="sdt")
        nc.gpsimd.dma_gather(
            sdt_t[:].rearrange("p (c e) -> p c e", e=srow),
            sdt[:, :], sdsb[:, soff // 16:(soff + nidx) // 16],
            nidx, nidx, srow, queue_num=(ci + 2) % 4)
        g3 = gt[:].rearrange("p (c e) -> p c e", e=row)
        s3 = sdt_t[:].rearrange("p (c e) -> p c e", e=srow)
        if AGG_LEVEL <= 1:
            st = stg.tile([128, prow], bf16, tag="st")
            nc.vector.tensor_copy(st[:, 0:prow], gt[:, 0:prow])
            nc.vector.tensor_copy(st[:, 0:srow // 2], sdt_t[:, 0:srow // 2])
            nc.sync.dma_start(pa[0:128, 0:prow], st[:, 0:prow])
            continue
        # one-hot [128e, nch, 128d]
        oh = ohb.tile([128, nch * 128], bf16, tag="oh")
        nc.vector.tensor_tensor(
            oh[:].rearrange("p (c d) -> p c d", d=128),
            iota_sb[:].unsqueeze(1).broadcast_to([128, nch, 128]),
            dcsb[:, coff:coff + nch].unsqueeze(2).broadcast_to([128, nch, 128]),
            mybir.AluOpType.is_equal)
        # escore = s_src (gathered row) + s_dst (s-gather)
        esc = escb.tile([128, nch * nh], f32, tag="esc")
        e3 = esc[:].rearrange("p (c h) -> p c h", h=nh)
        nc.vector.tensor_tensor(e3, g3[:, :, zw:zw + nh], s3[:, :, 0:nh],
                                mybir.AluOpType.add)
        nc.vector.scalar_tensor_tensor(esc[:], esc[:], 0.01, esc[:],
                                       mybir.AluOpType.mult,
                                       mybir.AluOpType.max)
        zs = zsb.tile([128, nch * zcol], bf16, tag="zs")
        z3 = zs[:].rearrange("p (c e) -> p c e", e=zcol)
        nc.scalar.activation(z3[:, :, zw:zcol], e3, Exp)
        # z * exp broadcast per head
        nc.vector.tensor_tensor(
            z3[:, :, 0:zw].rearrange("p c (h o) -> p c h o", h=nh),
            g3[:, :, 0:zw].rearrange("p c (h o) -> p c h o", h=nh),
            z3[:, :, zw:zcol].unsqueeze(3).broadcast_to(
                [128, nch, nh, zw // nh]),
            mybir.AluOpType.mult)
        if zw + nh <= 512:
            # single fused matmul per chunk: rhs [z | exp]
            for j in range(nch):
                t, first, last = cm[coff + j]
                if first:
                    cur_ph = ph.tile([128, zw + nh], f32, tag="ph")
                nc.tensor.matmul(cur_ph[:], oh[:, j * 128:(j + 1) * 128],
                                 zs[:, j * zcol:(j + 1) * zcol],
                                 start=first, stop=last)
                if last:
                    st = stg.tile([128, prow], bf16, tag="st")
                    nc.vector.tensor_copy(st[:, 0:zw + nh], cur_ph[:])
                    for sel, src_lo, ln, dst_lo in routing[t]:
                        nc.sync.dma_start(
                            (pa if sel == 0 else pb)[dst_lo:dst_lo + ln,
                                                     0:zw + nh],
                            st[src_lo:src_lo + ln, 0:zw + nh])
        else:
            # two contiguous accumulation passes per tile (h then den):
            # interleaving open PSUM groups on PE crashes HW.
            jt = 0
            while jt < nch:
                t = cm[coff + jt][0]
                span = 0
                while jt + span < nch and cm[coff + jt + span][0] == t:
                    span += 1
                cur_ph = ph.tile([128, zw], f32, tag="ph")
                cur_pd = pd.tile([128, nh], f32, tag="pd")
                for q in range(span):
                    j = jt + q
                    nc.tensor.matmul(cur_ph[:], oh[:, j * 128:(j + 1) * 128],
                                     zs[:, j * zcol:j * zcol + zw],
                                     start=(q == 0), stop=(q == span - 1))
                for q in range(span):
                    j = jt + q
                    nc.tensor.matmul(cur_pd[:], oh[:, j * 128:(j + 1) * 128],
                                     zs[:, j * zcol + zw:(j + 1) * zcol],
                                     start=(q == 0), stop=(q == span - 1))
                st = stg.tile([128, prow], bf16, tag="st")
                nc.vector.tensor_copy(st[:, 0:zw], cur_ph[:])
                nc.vector.tensor_copy(st[:, zw:zw + nh], cur_pd[:])
                for sel, src_lo, ln, dst_lo in routing[t]:
                    nc.sync.dma_start(
                        (pa if sel == 0 else pb)[dst_lo:dst_lo + ln,
                                                 0:zw + nh],
                        st[src_lo:src_lo + ln, 0:zw + nh])
                jt += span
        if hooks and ci in hooks:
            hooks[ci]()


STOP_STAGE = int(os.environ.get("GAT_STOP_STAGE", "9"))


def _split_rows(r0, rows, half):
    """Split local row range [r0, r0+rows) at the half boundary.

    Yields (sel, lo_in_tile, ln, offset_in_half_tensor)."""
    out = []
    r = r0
    while r < r0 + rows:
        sel = 0 if r < half else 1
        end = min(r0 + rows, half if sel == 0 else r0 + rows)
        ln = end - r
        out.append((sel, r - r0, ln, r if sel == 0 else r - half))
        r += ln
    return out



def _build(cfg):
    stop = STOP_STAGE
    REPEAT = cfg.get("repeat", 1)
    bf16 = mybir.dt.bfloat16
    f32 = mybir.dt.float32
    i16 = mybir.dt.int16
    NS, IMG, BLK = cfg["NS"], cfg["IMG"], cfg["BLK"]
    ND0, ND1 = cfg["ND0"], cfg["ND1"]
    SS, DS0, DS1 = cfg["SS"], cfg["DS0"], cfg["DS1"]
    S1, S2 = cfg["S1"], cfg["S2"]
    KI, KB, MB = IMG // 128, BLK // 128, F // 128
    O = F // H

    nc = bacc.Bacc("TRN2", target_bir_lowering=False, debug=False,
                   enable_asserts=True, num_devices=NCORE,
                   num_swdge_queues=4)

    def param(name, shape, dt):
        return nc.declare_dram_parameter(name, list(shape), dt, isOutput=False)

    imgT = param("imgT", [IMG, SS], bf16)
    blkT = param("blkT", [BLK, SS], bf16)
    wimgT = param("wimgT", [IMG, F], bf16)
    wblkT = param("wblkT", [BLK, F], bf16)
    wv = param("wv", [F, F], bf16)
    we = param("we", [F, F], bf16)
    fc1T = param("fc1T", [F, F], bf16)
    acat = param("acat", [F, 16], bf16)
    fc2T = param("fc2T", [F, OUT], bf16)
    a2s = param("a2s", [128, OUT], f32)
    a2d = param("a2d", [128, OUT], f32)
    biasv = param("biasv", [128, MB], f32)
    biase = param("biase", [128, MB], f32)
    iota = param("iota", [128, 128], f32)
    ident = param("ident", [128, 128], bf16)
    g1 = param("g1", [128, S1 // 16], i16)
    sd1 = param("sd1", [128, S1 // 16], i16)
    dc1 = param("dc1", [128, S1 // 128], f32)
    g2 = param("g2", [128, S2 // 16], i16)
    sd2 = param("sd2", [128, S2 // 16], i16)
    dc2 = param("dc2", [128, S2 // 128], f32)
    tick = param("tick", [128, 1], f32)
    out = nc.declare_dram_parameter("out", [DS1, OUT], f32, isOutput=True)
    tock = nc.declare_dram_parameter("tock", [128, 1], f32, isOutput=True)

    half1, half2 = cfg["half1"], cfg["half2"]
    table1 = nc.dram_tensor("table1", [SS, ROW1], bf16)
    sdsh1 = nc.dram_tensor("sdsh1", [SS, SROW], bf16)
    sdt1 = nc.dram_tensor("sdt1", [NS, SROW], bf16, addr_space="Shared")
    sdt1b = nc.dram_tensor("sdt1b", [NS, SROW], bf16)
    partials1a = nc.dram_tensor("partials1a", [NCORE * half1, PR1], bf16)
    partials1b = nc.dram_tensor("partials1b", [NCORE * half1, PR1], bf16)
    rs1a = nc.dram_tensor("rs1a", [half1, PR1], bf16)
    rs1b = nc.dram_tensor("rs1b", [half1, PR1], bf16)
    table2 = nc.dram_tensor("table2", [DS0, ROW2], bf16)
    sdsh2 = nc.dram_tensor("sdsh2", [DS0, SROW], bf16)
    sdt2 = nc.dram_tensor("sdt2", [ND0, SROW], bf16, addr_space="Shared")
    sdt2b = nc.dram_tensor("sdt2b", [ND0, SROW], bf16)
    partials2a = nc.dram_tensor("partials2a", [NCORE * half2, PR2], bf16)
    partials2b = nc.dram_tensor("partials2b", [NCORE * half2, PR2], bf16)
    rs2a = nc.dram_tensor("rs2a", [half2, PR2], bf16)
    rs2b = nc.dram_tensor("rs2b", [half2, PR2], bf16)

    Sig = mybir.ActivationFunctionType.Sigmoid
    Exp = mybir.ActivationFunctionType.Exp
    TT = nc.vector.tensor_tensor
    MUL = mybir.AluOpType.mult
    ADD = mybir.AluOpType.add

    from contextlib import ExitStack
    if True:
      with tile.TileContext(nc) as tc, ExitStack() as top:
        res = top.enter_context(tc.tile_pool(name="res", bufs=1))
        # resident weights / constants
        wimg_sb = res.tile([128, KI * F], bf16)
        nc.sync.dma_start(wimg_sb[:].rearrange("p (k m) -> p k m", k=KI),
                          wimgT[:, :].rearrange("(k p) m -> p k m", p=128))
        wblk_sb = res.tile([128, KB * F], bf16)
        nc.sync.dma_start(wblk_sb[:].rearrange("p (k m) -> p k m", k=KB),
                          wblkT[:, :].rearrange("(k p) m -> p k m", p=128))
        wv_sb = res.tile([128, MB * F], bf16)
        nc.sync.dma_start(wv_sb[:].rearrange("p (k m) -> p k m", k=MB),
                          wv[:, :].rearrange("(k p) m -> p k m", p=128))
        we_sb = res.tile([128, MB * F], bf16)
        nc.sync.dma_start(we_sb[:].rearrange("p (k m) -> p k m", k=MB),
                          we[:, :].rearrange("(k p) m -> p k m", p=128))
        fc1_sb = res.tile([128, MB * F], bf16)
        nc.sync.dma_start(fc1_sb[:].rearrange("p (k m) -> p k m", k=MB),
                          fc1T[:, :].rearrange("(k p) m -> p k m", p=128))
        acat_sb = res.tile([128, MB * 16], bf16)
        nc.sync.dma_start(acat_sb[:].rearrange("p (k m) -> p k m", k=MB),
                          acat[:, :].rearrange("(k p) m -> p k m", p=128))
        fc2_sb = res.tile([128, MB * OUT], bf16)
        nc.sync.dma_start(fc2_sb[:].rearrange("p (k m) -> p k m", k=MB),
                          fc2T[:, :].rearrange("(k p) m -> p k m", p=128))
        a2s_sb = res.tile([128, OUT], f32)
        nc.sync.dma_start(a2s_sb[:], a2s[:, :])
        a2d_sb = res.tile([128, OUT], f32)
        nc.sync.dma_start(a2d_sb[:], a2d[:, :])
        bv_sb = res.tile([128, MB], f32)
        nc.sync.dma_start(bv_sb[:], biasv[:, :])
        be_sb = res.tile([128, MB], f32)
        nc.sync.dma_start(be_sb[:], biase[:, :])
        iota_sb = res.tile([128, 128], f32)
        nc.sync.dma_start(iota_sb[:], iota[:, :])
        id_sb = res.tile([128, 128], bf16)
        nc.sync.dma_start(id_sb[:], ident[:, :])
        g1_sb = res.tile([128, S1 // 16], i16)
        nc.sync.dma_start(g1_sb[:], g1[:, :])
        sd1_sb = res.tile([128, S1 // 16], i16)
        nc.sync.dma_start(sd1_sb[:], sd1[:, :])
        dc1_sb = res.tile([128, S1 // 128], f32)
        nc.sync.dma_start(dc1_sb[:], dc1[:, :])
        g2_sb = res.tile([128, S2 // 16], i16)
        nc.sync.dma_start(g2_sb[:], g2[:, :])
        sd2_sb = res.tile([128, S2 // 16], i16)
        nc.sync.dma_start(sd2_sb[:], sd2[:, :])
        dc2_sb = res.tile([128, S2 // 128], f32)
        nc.sync.dma_start(dc2_sb[:], dc2[:, :])

        # chain for timing
        tk = res.tile([128, 1], f32)
        nc.sync.dma_start(tk[:], tick[:, :])
        nc.sync.dma_start(tock[:, :], tk[:])
        if stop < 9:
            zo = res.tile([128, OUT], f32)
            nc.vector.memset(zo[:], 0.0)
            for tt in range(_ceil(DS1, 128)):
                rows = min(128, DS1 - tt * 128)
                nc.sync.dma_start(out[tt * 128:tt * 128 + rows, :],
                                  zo[:rows, :])

        for _rep in range(REPEAT):
          if _rep:
              # serialize repeats so repeat-K timing measures a full
              # dependency-honest iteration (idempotent reps would
              # otherwise overlap through untracked DRAM reuse)
              tc.strict_bb_all_engine_barrier()
          # ---------------- Phase A ----------------
          WA = min(500, SS)
          with ExitStack() as pa:
              rhsp = pa.enter_context(tc.tile_pool(name=f"parhs{_rep}", bufs=2))
              sbp = pa.enter_context(tc.tile_pool(name=f"pasb{_rep}", bufs=2))
              psp = pa.enter_context(tc.tile_pool(name=f"paps{_rep}", bufs=4, space="PSUM"))
              pst = pa.enter_context(tc.tile_pool(name=f"patr{_rep}", bufs=2, space="PSUM"))
              stp = pa.enter_context(tc.tile_pool(name=f"past{_rep}", bufs=3))
              for nt in range(_ceil(SS, WA)):
                  n0 = nt * WA
                  w = min(WA, SS - n0)
                  x_sb = rhsp.tile([128, KI * w], bf16, tag="x")
                  nc.sync.dma_start(
                      x_sb[:].rearrange("p (k n) -> p k n", k=KI),
                      imgT[:, n0:n0 + w].rearrange("(k p) n -> p k n", p=128))
                  b_sb = rhsp.tile([128, KB * w], bf16, tag="b")
                  nc.sync.dma_start(
                      b_sb[:].rearrange("p (k n) -> p k n", k=KB),
                      blkT[:, n0:n0 + w].rearrange("(k p) n -> p k n", p=128))

                  def mm(lhs_sb, rhs_sb, K, m, width):
                      ps = psp.tile([128, width], f32, tag="ps")
                      for k in range(K):
                          nc.tensor.matmul(
                              ps[:],
                              lhs_sb[:, (k * F + m * 128):(k * F + m * 128) + 128],
                              rhs_sb[:, k * width:(k + 1) * width],
                              start=(k == 0), stop=(k == K - 1))
                      return ps

                  fi_sb = sbp.tile([128, MB * w], bf16, tag="fi")
                  ti_sb = sbp.tile([128, MB * w], bf16, tag="ti")
                  av_sb = sbp.tile([128, MB * w], bf16, tag="av")
                  ae_sb = sbp.tile([128, MB * w], bf16, tag="ae")
                  for m in range(MB):
                      ps = mm(wimg_sb, x_sb, KI, m, w)
                      nc.vector.tensor_copy(fi_sb[:, m * w:(m + 1) * w], ps[:])
                  for m in range(MB):
                      ps = mm(wblk_sb, b_sb, KB, m, w)
                      nc.vector.tensor_copy(ti_sb[:, m * w:(m + 1) * w], ps[:])
                  for m in range(MB):
                      ps = mm(wv_sb, fi_sb, MB, m, w)
                      nc.scalar.activation(av_sb[:, m * w:(m + 1) * w], ps[:],
                                           Sig, bias=bv_sb[:, m:m + 1])
                  for m in range(MB):
                      ps = mm(we_sb, ti_sb, MB, m, w)
                      nc.scalar.activation(ae_sb[:, m * w:(m + 1) * w], ps[:],
                                           Sig, bias=be_sb[:, m:m + 1])
                  fu_sb = sbp.tile([128, MB * w], bf16, tag="fu")
                  TT(fu_sb[:], av_sb[:], fi_sb[:], MUL)
                  TT(ae_sb[:], ae_sb[:], ti_sb[:], MUL)
                  TT(fu_sb[:], fu_sb[:], ae_sb[:], ADD)
                  z_sb = sbp.tile([128, MB * w], bf16, tag="z")
                  for m in range(MB):
                      ps = mm(fc1_sb, fu_sb, MB, m, w)
                      nc.vector.tensor_copy(z_sb[:, m * w:(m + 1) * w], ps[:])
                  pss = psp.tile([128, w], f32, tag="ps")
                  for k in range(MB):
                      nc.tensor.matmul(pss[:16, :], acat_sb[:, k * 16:(k + 1) * 16],
                                       fu_sb[:, k * w:(k + 1) * w],
                                       start=(k == 0), stop=(k == MB - 1))
                  s_sb = sbp.tile([16, w], bf16, tag="s")
                  nc.vector.tensor_copy(s_sb[:], pss[:16, :])
                  for b0 in range(0, w, 128):
                      wb = min(128, w - b0)
                      st = stp.tile([128, ROW1], bf16, tag="t1")
                      nc.vector.memset(st[:, F + 16:ROW1], 0.0)
                      for m in range(MB):
                          ptr = pst.tile([128, 128], bf16, tag="tr")
                          nc.tensor.matmul(ptr[:wb, :],
                                           z_sb[:, m * w + b0:m * w + b0 + wb],
                                           id_sb[:], is_transpose=True)
                          nc.vector.tensor_copy(
                              st[:wb, m * 128:(m + 1) * 128], ptr[:wb, :])
                      ptr = pst.tile([128, 128], bf16, tag="tr")
                      nc.tensor.matmul(ptr[:wb, :16], s_sb[:, b0:b0 + wb],
                                       id_sb[:16, :16], is_transpose=True)
                      nc.vector.tensor_copy(st[:wb, F:F + 16], ptr[:wb, :16])
                      nc.sync.dma_start(table1[n0 + b0:n0 + b0 + wb, :],
                                        st[:wb, :])
                      st2 = stp.tile([128, SROW], bf16, tag="t2")
                      nc.vector.memset(st2[:, 8:SROW], 0.0)
                      nc.vector.tensor_copy(st2[:wb, 0:8], st[:wb, F + 8:F + 16])
                      nc.sync.dma_start(sdsh1[n0 + b0:n0 + b0 + wb, :],
                                        st2[:wb, :])

          if stop >= 2:
              nc.gpsimd.collective_compute(
                  "AllGather", mybir.AluOpType.bypass,
                  replica_groups=[list(range(NCORE))],
                  ins=[sdsh1[:, :]], outs=[sdt1[:, :]])
              nc.sync.dma_start(sdt1b[:, :], sdt1[:, :])

          # ---------------- Layer-1 aggregation ----------------
          def _rs1a():
              nc.gpsimd.collective_compute(
                  "ReduceScatter", ADD, replica_groups=[list(range(NCORE))],
                  ins=[partials1a[:, :]], outs=[rs1a[:, :]])

          with ExitStack() as ag1:
            if stop >= 3:
              _agg_layer(nc, tc, ag1, cfg, table=table1, sdt=sdt1b,
                         gsb=g1_sb, sdsb=sd1_sb, dcsb=dc1_sb, iota_sb=iota_sb,
                         calls=cfg["calls1"], cm=cfg["cm1"],
                         row=ROW1, srow=SROW, prow=PR1, zw=F, nh=H,
                         pa=partials1a, pb=partials1b, routing=cfg["rt1"],
                         hooks=({cfg["sp1"]: _rs1a} if stop >= 4 else None),
                         rep=_rep)

          if stop >= 4:
              nc.gpsimd.collective_compute(
                  "ReduceScatter", ADD, replica_groups=[list(range(NCORE))],
                  ins=[partials1b[:, :]], outs=[rs1b[:, :]])

          # ---------------- normalize + layer-2 tables ----------------
          with ExitStack() as p4:
            if stop >= 5:
              sbp = p4.enter_context(tc.tile_pool(name=f"n2sb{_rep}", bufs=3))
              psp = p4.enter_context(tc.tile_pool(name=f"n2ps{_rep}", bufs=2, space="PSUM"))
              ptp = p4.enter_context(tc.tile_pool(name=f"n2pt{_rep}", bufs=2, space="PSUM"))
              for tt in range(_ceil(DS0, 128)):
                  r0 = tt * 128
                  rows = min(128, DS0 - r0)
                  hs = sbp.tile([128, PR1], bf16, tag="hs")
                  for (h_sel, h_lo, h_ln, h_dst) in _split_rows(
                          r0, rows, half1):
                      nc.sync.dma_start(
                          hs[h_lo:h_lo + h_ln, :],
                          (rs1a if h_sel == 0 else rs1b)[
                              h_dst:h_dst + h_ln, :])
                  rden = sbp.tile([128, H], f32, tag="rd")
                  nc.vector.reciprocal(rden[:rows, :], hs[:rows, F:F + H])
                  hraw = sbp.tile([128, F], f32, tag="hraw")
                  TT(hraw[:rows, :].rearrange("p (h o) -> p h o", h=H),
                     hs[:rows, 0:F].rearrange("p (h o) -> p h o", h=H),
                     rden[:rows, :].unsqueeze(2).broadcast_to(
                         [rows, H, F // H]),
                     MUL)
                  t1 = sbp.tile([128, F], f32, tag="t1")
                  nc.vector.tensor_scalar_min(t1[:rows, :], hraw[:rows, :], 0.0)
                  nc.scalar.activation(t1[:rows, :], t1[:rows, :], Exp)
                  h1 = sbp.tile([128, F], bf16, tag="h1")
                  nc.vector.scalar_tensor_tensor(
                      h1[:rows, :], t1[:rows, :], -1.0, hraw[:rows, :],
                      ADD, mybir.AluOpType.max)
                  h1t = sbp.tile([128, MB * 128], bf16, tag="h1t")
                  for m in range(MB):
                      ptr = ptp.tile([128, 128], bf16, tag="tr")
                      nc.tensor.matmul(ptr[:, :rows],
                                       h1[:rows, m * 128:(m + 1) * 128],
                                       id_sb[:rows, :rows], is_transpose=True)
                      nc.vector.tensor_copy(h1t[:, m * 128:m * 128 + rows],
                                            ptr[:, :rows])
                  pz2 = psp.tile([128, OUT], f32, tag="z2")
                  for k in range(MB):
                      nc.tensor.matmul(pz2[:rows, :],
                                       h1t[:, k * 128:k * 128 + rows],
                                       fc2_sb[:, k * OUT:(k + 1) * OUT],
                                       start=(k == 0), stop=(k == MB - 1))
                  scr = sbp.tile([128, OUT], f32, tag="scr")
                  s2s = sbp.tile([128, 1], f32, tag="s2s")
                  s2d = sbp.tile([128, 1], f32, tag="s2d")
                  if P4_LEVEL >= 3:
                      nc.vector.tensor_tensor_reduce(
                          scr[:rows, :], pz2[:rows, :], a2s_sb[:rows, :], 1.0,
                          0.0, MUL, ADD, s2s[:rows, :])
                      nc.vector.tensor_tensor_reduce(
                          scr[:rows, :], pz2[:rows, :], a2d_sb[:rows, :], 1.0,
                          0.0, MUL, ADD, s2d[:rows, :])
                  else:
                      nc.vector.tensor_tensor(scr[:rows, :], pz2[:rows, :],
                                              a2s_sb[:rows, :], MUL)
                      nc.vector.reduce_sum(s2s[:rows, :], scr[:rows, :],
                                           mybir.AxisListType.X)
                      nc.vector.tensor_tensor(scr[:rows, :], pz2[:rows, :],
                                              a2d_sb[:rows, :], MUL)
                      nc.vector.reduce_sum(s2d[:rows, :], scr[:rows, :],
                                           mybir.AxisListType.X)
                  st = sbp.tile([128, ROW2], bf16, tag="st")
                  nc.vector.memset(st[:, OUT + 1:ROW2], 0.0)
                  nc.vector.tensor_copy(st[:rows, 0:OUT], pz2[:rows, :])
                  nc.vector.tensor_copy(st[:rows, OUT:OUT + 1], s2s[:rows, :])
                  nc.sync.dma_start(table2[r0:r0 + rows, :], st[:rows, :])
                  st2 = sbp.tile([128, SROW], bf16, tag="st2")
                  nc.vector.memset(st2[:, 1:SROW], 0.0)
                  nc.vector.tensor_copy(st2[:rows, 0:1], s2d[:rows, :])
                  nc.sync.dma_start(sdsh2[r0:r0 + rows, :], st2[:rows, :])

          if stop >= 6:
              nc.gpsimd.collective_compute(
                  "AllGather", mybir.AluOpType.bypass,
                  replica_groups=[list(range(NCORE))],
                  ins=[sdsh2[:, :]], outs=[sdt2[:, :]])
              nc.sync.dma_start(sdt2b[:, :], sdt2[:, :])

          # ---------------- Layer-2 aggregation ----------------
          def _rs2a():
              nc.gpsimd.collective_compute(
                  "ReduceScatter", ADD, replica_groups=[list(range(NCORE))],
                  ins=[partials2a[:, :]], outs=[rs2a[:, :]])

          with ExitStack() as ag2:
            if stop >= 7:
              _agg_layer(nc, tc, ag2, cfg, table=table2, sdt=sdt2b,
                         gsb=g2_sb, sdsb=sd2_sb, dcsb=dc2_sb, iota_sb=iota_sb,
                         calls=cfg["calls2"], cm=cfg["cm2"],
                         row=ROW2, srow=SROW, prow=PR2, zw=OUT, nh=1,
                         pa=partials2a, pb=partials2b, routing=cfg["rt2"],
                         hooks=({cfg["sp2"]: _rs2a} if stop >= 8 else None),
                         rep=_rep)

          if stop >= 8:
              nc.gpsimd.collective_compute(
                  "ReduceScatter", ADD, replica_groups=[list(range(NCORE))],
                  ins=[partials2b[:, :]], outs=[rs2b[:, :]])

          # ---------------- final normalize ----------------
          with ExitStack() as p8:
            if stop >= 9:
              sbp = p8.enter_context(tc.tile_pool(name=f"fsb{_rep}", bufs=3))
              for tt in range(_ceil(DS1, 128)):
                  r0 = tt * 128
                  rows = min(128, DS1 - r0)
                  hs = sbp.tile([128, PR2], bf16, tag="hs")
                  for (h_sel, h_lo, h_ln, h_dst) in _split_rows(
                          r0, rows, half2):
                      nc.sync.dma_start(
                          hs[h_lo:h_lo + h_ln, :],
                          (rs2a if h_sel == 0 else rs2b)[
                              h_dst:h_dst + h_ln, :])
                  rden = sbp.tile([128, 1], f32, tag="rd")
                  nc.vector.reciprocal(rden[:rows, :], hs[:rows, OUT:OUT + 1])
                  ot = sbp.tile([128, OUT], f32, tag="ot")
                  TT(ot[:rows, :], hs[:rows, 0:OUT],
                     rden[:rows, :].broadcast_to([rows, OUT]), MUL)
                  nc.sync.dma_start(out[r0:r0 + rows, :], ot[:rows, :])

    nc.compile()
    return nc


_CACHE = {}


def _get_nc(cfg):
    key = repr(sorted((k, v) for k, v in cfg.items()))
    if key not in _CACHE:
        _CACHE[key] = _build(cfg)
    return _CACHE[key]


def kernel(**inputs) -> np.ndarray:
    cfg, in_maps = _prep(inputs)
    nc = _get_nc(cfg)
    res = run_bass_kernel_spmd(nc, in_maps, core_ids=list(range(NCORE)))
    return np.concatenate([res.results[c]["out"] for c in range(NCORE)],
                          axis=0)



# revision 15
# speedup vs baseline: 1.1294x; 1.0768x over previous
"""Trainium2 Bass kernel for a 2-layer GAT (cross-attention fusion + 8-head GAT
+ 1-head GAT) distributed over 8 NeuronCores.

Strategy (src-sharded message passing, all gathers local):
  - Phase A: per-node feature transforms sharded by src node (NS/8 per core),
    activations kept transposed [feat, node] so matmuls contract on partitions.
    Each core writes a local gather-table row per owned node:
      [z(512) | s_src(8) | s_dst(8) | pad] bf16, 1280B rows.
  - A small AllGather replicates a padded per-node s_dst table (256B rows).
  - Layer-1 aggregation: each core processes the edges whose src it owns.
    dma_gather fetches z rows from the LOCAL table (int16 idx ok), a second
    gather fetches s_dst by global dst. Per-edge exp(leakyrelu(s_src+s_dst))
    weights are applied, and one-hot matmuls accumulate exp-weighted partial
    sums + denominators per global dst tile in PSUM.
  - One bf16 ReduceScatter sums partials [ND0, 520] across cores; each core
    ends with its own dst range = its layer-2 src shard.
  - Normalize + ELU + z2 projection per owned node -> local layer-2 table,
    then the same aggregation scheme for layer 2 and a final ReduceScatter.
"""
import os
import sys
import math

sys.path.insert(0, "/opt/trn_rl_repo")

import numpy as np
import ml_dtypes

import concourse.bass as bass
import concourse.bacc as bacc
import concourse.tile as tile
import concourse.mybir as mybir
from concourse.bass_utils import run_bass_kernel_spmd

BF16 = ml_dtypes.bfloat16
NCORE = 8
F = 512          # fused dim
H = 8            # layer-1 heads
OUT = 128        # layer-2 out dim
ROW1 = 640       # layer-1 table row, bf16 elems (1280B): [z 512|s_src 8|s_dst 8|pad]
SROW = 128       # s_dst table row, bf16 elems (256B)
ROW2 = 256       # layer-2 table row (512B): [z2 128|s2src 1|pad]
PR1 = 520        # layer-1 partial row: [h 512 | den 8]
PR2 = 129        # layer-2 partial row: [h 128 | den 1]
BUDGET_CHUNKS = 4    # chunks per dma_gather call (4 SWDGE queues in flight)


def _ceil(a, b):
    return -(-a // b)


def _wrap_idx(idx):
    """[S] -> [128, S//16] int16, wrapped in 16 partitions, replicated 8x."""
    w = idx.reshape(-1, 16).T.astype(np.int16)
    return np.ascontiguousarray(np.tile(w, (8, 1)))


def _routing(t, n_dst, own, half):
    """Store routing for dst tile t: list of (sel, src_lo, ln, dst_lo).

    Global partial rows are re-laid-out as two half tensors so a
    ReduceScatter over each delivers core c exactly the first/second half
    of its owned range: half-0 rows = concat_c [own*c, own*c+half),
    half-1 rows = concat_c [own*c+half, own*(c+1))."""
    r0, r1 = 128 * t, min(128 * (t + 1), n_dst)
    out = []
    r = r0
    while r < r1:
        c, loc = r // own, r % own
        sel = 0 if loc < half else 1
        seg_end = own * c + (half if sel == 0 else own)
        ln = min(r1, seg_end) - r
        dst = half * c + (loc if sel == 0 else loc - half)
        out.append((sel, r - r0, ln, dst))
        r += ln
    return tuple(out)


def _sched(src, dst, shard, n_dst, own):
    """Static per-dst-tile schedule, uniform across cores.

    Tiles are processed group-A-first (tiles containing any first-half
    row of a core's owned range) so the half-0 partials tensor is
    complete mid-aggregation and its ReduceScatter overlaps the rest.

    Returns (chunkmeta, calls, S, g[8,S], sd[8,S], dcol[8,S],
             routing, split_call, half)."""
    T = _ceil(n_dst, 128)
    half = own // 2
    owner = src // shard
    percore = []
    cnt = np.zeros((NCORE, T), np.int64)
    for c in range(NCORE):
        m = owner == c
        s_loc = (src[m] - c * shard).astype(np.int64)
        d = dst[m].astype(np.int64)
        o = np.argsort(d, kind="stable")
        s_loc, d = s_loc[o], d[o]
        t = d // 128
        cnt[c] = np.bincount(t, minlength=T)
        percore.append((s_loc, d, t))
    C = np.maximum(1, _ceil_arr(cnt.max(axis=0), 128))
    routing = {t: _routing(t, n_dst, own, half) for t in range(T)}
    grpA = [t for t in range(T) if any(s == 0 for s, _, _, _ in routing[t])]
    grpB = [t for t in range(T) if t not in set(grpA)]
    perm = grpA + grpB
    chunkmeta = []
    tile_base = np.zeros(T, np.int64)
    off = 0
    nchunks_A = 0
    for t in perm:
        tile_base[t] = off
        for p in range(int(C[t])):
            chunkmeta.append((t, p == 0, p == C[t] - 1))
        off += C[t] * 128
        if t in set(grpA):
            nchunks_A += int(C[t])
    S = int(off)
    calls = []
    cur_off = cur_n = 0
    for t in perm:
        if cur_n + C[t] > BUDGET_CHUNKS and cur_n > 0:
            calls.append((cur_off, cur_n))
            cur_off += cur_n
            cur_n = 0
        cur_n += int(C[t])
    calls.append((cur_off, cur_n))
    # call whose chunks complete group A -> hook point for the first RS
    split_call = 0
    acc = 0
    for i, (coff, n) in enumerate(calls):
        acc += n
        if acc >= nchunks_A:
            split_call = i
            break
    g = np.zeros((NCORE, S), np.int64)
    sd = np.zeros((NCORE, S), np.int64)
    dcol = np.full((NCORE, S), -1.0, np.float32)
    for c in range(NCORE):
        s_loc, d, t = percore[c]
        starts = np.searchsorted(t, np.arange(T))
        pos = np.arange(len(t)) - starts[t]
        slot = tile_base[t] + pos
        g[c, slot] = s_loc
        sd[c, slot] = d
        dcol[c, slot] = (d - t * 128).astype(np.float32)
    routing_t = tuple(routing[t] for t in range(T))
    return chunkmeta, calls, S, g, sd, dcol, routing_t, split_call, half


def _ceil_arr(a, b):
    return -(-a // b)


def _prep(inputs):
    img = np.asarray(inputs["image_features"], np.float32)
    blk = np.asarray(inputs["block_features"], np.float32)
    W_img = np.asarray(inputs["W_img"], np.float32)
    W_blk = np.asarray(inputs["W_blk"], np.float32)
    Wv = np.asarray(inputs["Wv"], np.float32)
    bv = np.asarray(inputs["bv"], np.float32)
    We = np.asarray(inputs["We"], np.float32)
    be = np.asarray(inputs["be"], np.float32)
    fc1 = np.asarray(inputs["fc1"], np.float32)
    attn1 = np.asarray(inputs["attn1"], np.float32)
    fc2 = np.asarray(inputs["fc2"], np.float32)
    attn2 = np.asarray(inputs["attn2"], np.float32)
    e0s = np.asarray(inputs["edge0_src"], np.int64)
    e0d = np.asarray(inputs["edge0_dst"], np.int64)
    e1s = np.asarray(inputs["edge1_src"], np.int64)
    e1d = np.asarray(inputs["edge1_dst"], np.int64)
    ND0 = int(inputs["n_dst0"])
    ND1 = int(inputs["n_dst1"])

    NS, IMG = img.shape
    BLK = blk.shape[1]
    assert W_img.shape == (F, IMG) and W_blk.shape == (F, BLK)
    assert fc1.shape == (H, F // H, F) and fc2.shape[1] == OUT
    assert NS % NCORE == 0 and ND0 % NCORE == 0 and ND1 % NCORE == 0
    assert IMG % 128 == 0 and BLK % 128 == 0
    SS, DS0, DS1 = NS // NCORE, ND0 // NCORE, ND1 // NCORE

    O = F // H
    # host-derived weights
    wimgT = np.ascontiguousarray(W_img.T).astype(BF16)          # [IMG, F]
    wblkT = np.ascontiguousarray(W_blk.T).astype(BF16)          # [BLK, F]
    wv = Wv.astype(BF16)                                        # [F, F] lhsT
    we = We.astype(BF16)
    fc1T = np.ascontiguousarray(fc1.reshape(F, F).T).astype(BF16)   # [F, (h o)]
    a_src = np.einsum("hof,ho->fh", fc1, attn1[:, :O])
    a_dst = np.einsum("hof,ho->fh", fc1, attn1[:, O:])
    acat = np.concatenate([a_src, a_dst], axis=1).astype(BF16)  # [F, 16]
    fc2T = np.ascontiguousarray(fc2[0].T).astype(BF16)          # [F, OUT]
    a2s = np.tile(attn2[0, :OUT].astype(np.float32), (128, 1))  # [128, OUT]
    a2d = np.tile(attn2[0, OUT:].astype(np.float32), (128, 1))
    MB = F // 128
    biasv = np.ascontiguousarray(bv.reshape(MB, 128).T).astype(np.float32)
    biase = np.ascontiguousarray(be.reshape(MB, 128).T).astype(np.float32)
    iota = np.tile(np.arange(128, dtype=np.float32), (128, 1))
    ident = np.eye(128, dtype=np.float32).astype(BF16)

    (cm1, calls1, S1, g1, sd1, dc1,
     rt1, sp1, half1) = _sched(e0s, e0d, SS, ND0, ND0 // NCORE)
    (cm2, calls2, S2, g2, sd2, dc2,
     rt2, sp2, half2) = _sched(e1s, e1d, DS0, ND1, ND1 // NCORE)

    shared = dict(wimgT=wimgT, wblkT=wblkT, wv=wv, we=we, fc1T=fc1T, acat=acat,
                  fc2T=fc2T, a2s=a2s, a2d=a2d, biasv=biasv, biase=biase,
                  iota=iota, ident=ident,
                  tick=np.zeros((128, 1), np.float32))
    in_maps = []
    for c in range(NCORE):
        m = dict(shared)
        m["imgT"] = np.ascontiguousarray(
            img[c * SS:(c + 1) * SS].T).astype(BF16)
        m["blkT"] = np.ascontiguousarray(
            blk[c * SS:(c + 1) * SS].T).astype(BF16)
        m["g1"] = _wrap_idx(g1[c])
        m["sd1"] = _wrap_idx(sd1[c])
        m["dc1"] = np.ascontiguousarray(dc1[c].reshape(-1, 128).T)
        m["g2"] = _wrap_idx(g2[c])
        m["sd2"] = _wrap_idx(sd2[c])
        m["dc2"] = np.ascontiguousarray(dc2[c].reshape(-1, 128).T)
        in_maps.append(m)

    cfg = dict(NS=NS, IMG=IMG, BLK=BLK, ND0=ND0, ND1=ND1,
               SS=SS, DS0=DS0, DS1=DS1,
               cm1=tuple(cm1), calls1=tuple(calls1), S1=S1,
               cm2=tuple(cm2), calls2=tuple(calls2), S2=S2,
               rt1=rt1, sp1=sp1, half1=half1,
               rt2=rt2, sp2=sp2, half2=half2)
    return cfg, in_maps


# ---------------------------------------------------------------- device code

AGG_LEVEL = int(os.environ.get("GAT_AGG_LEVEL", "3"))
P4_LEVEL = int(os.environ.get("GAT_P4_LEVEL", "2"))


def _agg_layer(nc, tc, ctx, cfg, *, table, sdt, gsb, sdsb, dcsb, iota_sb,
               calls, cm, row, srow, prow, zw, nh, pa, pb, routing,
               hooks=None, rep=0):
    """Shared edge-aggregation pipeline for both GAT layers.

    row: gather row width (elems); srow: s_dst row width; prow: partial row;
    zw: z width (512 or 128); nh: heads (8 or 1).
    Layer-1 rows: [z 512 | s_src 8 | s_dst 8 | pad]; s-gather gives s_dst.
    Layer-2 rows: [z2 128 | s2src 1 | pad]; s-gather gives s2dst.
    """
    bf16 = mybir.dt.bfloat16
    f32 = mybir.dt.float32
    maxch = max(n for _, n in calls)
    gb = ctx.enter_context(tc.tile_pool(name=f"gb{zw}_{rep}", bufs=6))
    sdb = ctx.enter_context(tc.tile_pool(name=f"sdb{zw}_{rep}", bufs=6))
    ohb = ctx.enter_context(tc.tile_pool(name=f"ohb{zw}_{rep}", bufs=2))
    zsb = ctx.enter_context(tc.tile_pool(name=f"zsb{zw}_{rep}", bufs=2))
    escb = ctx.enter_context(tc.tile_pool(name=f"escb{zw}_{rep}", bufs=2))
    stg = ctx.enter_context(tc.tile_pool(name=f"stg{zw}_{rep}", bufs=3))
    ph = ctx.enter_context(tc.tile_pool(name=f"ph{zw}_{rep}", bufs=2, space="PSUM"))
    pd = ctx.enter_context(tc.tile_pool(name=f"pd{zw}_{rep}", bufs=2, space="PSUM"))

    zcol = zw + nh  # matmul rhs width: [z | exp]
    Exp = mybir.ActivationFunctionType.Exp
    cur_ph = cur_pd = None
    for ci, (coff, nch) in enumerate(calls):
        soff = coff * 128
        nidx = nch * 128
        gt = gb.tile([128, nch * row], bf16, tag="gt")
        nc.gpsimd.dma_gather(
            gt[:].rearrange("p (c e) -> p c e", e=row),
            table[:, :], gsb[:, soff // 16:(soff + nidx) // 16],
            nidx, nidx, row, queue_num=ci % 4)
        sdt_t = sdb.tile([128, nch * srow], bf16, tag="sdt")
        nc.gpsimd.dma_gather(
            sdt_t[:].rearrange("p (c e) -> p c e", e=srow),
            sdt[:, :], sdsb[:, soff // 16:(soff + nidx) // 16],
            nidx, nidx, srow, queue_num=(ci + 2) % 4)
        g3 = gt[:].rearrange("p (c e) -> p c e", e=row)
        s3 = sdt_t[:].rearrange("p (c e) -> p c e", e=srow)
        if AGG_LEVEL <= 1:
            st = stg.tile([128, prow], bf16, tag="st")
            nc.vector.tensor_copy(st[:, 0:prow], gt[:, 0:prow])
            nc.vector.tensor_copy(st[:, 0:srow // 2], sdt_t[:, 0:srow // 2])
            nc.sync.dma_start(pa[0:128, 0:prow], st[:, 0:prow])
            continue
        # one-hot [128e, nch, 128d]
        oh = ohb.tile([128, nch * 128], bf16, tag="oh")
        nc.vector.tensor_tensor(
            oh[:].rearrange("p (c d) -> p c d", d=128),
            iota_sb[:].unsqueeze(1).broadcast_to([128, nch, 128]),
            dcsb[:, coff:coff + nch].unsqueeze(2).broadcast_to([128, nch, 128]),
            mybir.AluOpType.is_equal)
        # escore = s_src (gathered row) + s_dst (s-gather)
        esc = escb.tile([128, nch * nh], f32, tag="esc")
        e3 = esc[:].rearrange("p (c h) -> p c h", h=nh)
        nc.vector.tensor_tensor(e3, g3[:, :, zw:zw + nh], s3[:, :, 0:nh],
                                mybir.AluOpType.add)
        nc.vector.scalar_tensor_tensor(esc[:], esc[:], 0.01, esc[:],
                                       mybir.AluOpType.mult,
                                       mybir.AluOpType.max)
        zs = zsb.tile([128, nch * zcol], bf16, tag="zs")
        z3 = zs[:].rearrange("p (c e) -> p c e", e=zcol)
        nc.scalar.activation(z3[:, :, zw:zcol], e3, Exp)
        # z * exp broadcast per head
        nc.vector.tensor_tensor(
            z3[:, :, 0:zw].rearrange("p c (h o) -> p c h o", h=nh),
            g3[:, :, 0:zw].rearrange("p c (h o) -> p c h o", h=nh),
            z3[:, :, zw:zcol].unsqueeze(3).broadcast_to(
                [128, nch, nh, zw // nh]),
            mybir.AluOpType.mult)
        if zw + nh <= 512:
            # single fused matmul per chunk: rhs [z | exp]
            for j in range(nch):
                t, first, last = cm[coff + j]
                if first:
                    cur_ph = ph.tile([128, zw + nh], f32, tag="ph")
                nc.tensor.matmul(cur_ph[:], oh[:, j * 128:(j + 1) * 128],
                                 zs[:, j * zcol:(j + 1) * zcol],
                                 start=first, stop=last)
                if last:
                    st = stg.tile([128, prow], bf16, tag="st")
                    nc.vector.tensor_copy(st[:, 0:zw + nh], cur_ph[:])
                    for sel, src_lo, ln, dst_lo in routing[t]:
                        nc.sync.dma_start(
                            (pa if sel == 0 else pb)[dst_lo:dst_lo + ln,
                                                     0:zw + nh],
                            st[src_lo:src_lo + ln, 0:zw + nh])
        else:
            # two contiguous accumulation passes per tile (h then den):
            # interleaving open PSUM groups on PE crashes HW.
            jt = 0
            while jt < nch:
                t = cm[coff + jt][0]
                span = 0
                while jt + span < nch and cm[coff + jt + span][0] == t:
                    span += 1
                cur_ph = ph.tile([128, zw], f32, tag="ph")
                cur_pd = pd.tile([128, nh], f32, tag="pd")
                for q in range(span):
                    j = jt + q
                    nc.tensor.matmul(cur_ph[:], oh[:, j * 128:(j + 1) * 128],
                                     zs[:, j * zcol:j * zcol + zw],
                                     start=(q == 0), stop=(q == span - 1))
                for q in range(span):
                    j = jt + q
                    nc.tensor.matmul(cur_pd[:], oh[:, j * 128:(j + 1) * 128],
                                     zs[:, j * zcol + zw:(j + 1) * zcol],
                                     start=(q == 0), stop=(q == span - 1))
                st = stg.tile([128, prow], bf16, tag="st")
                nc.vector.tensor_copy(st[:, 0:zw], cur_ph[:])
                nc.vector.tensor_copy(st[:, zw:zw + nh], cur_pd[:])
                for sel, src_lo, ln, dst_lo in routing[t]:
                    nc.sync.dma_start(
                        (pa if sel == 0 else pb)[dst_lo:dst_lo + ln,
                                                 0:zw + nh],
                        st[src_lo:src_lo + ln, 0:zw + nh])
                jt += span
        if hooks and ci in hooks:
            hooks[ci]()


STOP_STAGE = int(os.environ.get("GAT_STOP_STAGE", "9"))


def _split_rows(r0, rows, half):
    """Split local row range [r0, r0+rows) at the half boundary.

    Yields (sel, lo_in_tile, ln, offset_in_half_tensor)."""
    out = []
    r = r0
    while r < r0 + rows:
        sel = 0 if r < half else 1
        end = min(r0 + rows, half if sel == 0 else r0 + rows)
        ln = end - r
        out.append((sel, r - r0, ln, r if sel == 0 else r - half))
        r += ln
    return out



def _build(cfg):
    stop = STOP_STAGE
    REPEAT = cfg.get("repeat", 1)
    bf16 = mybir.dt.bfloat16
    f32 = mybir.dt.float32
    i16 = mybir.dt.int16
    NS, IMG, BLK = cfg["NS"], cfg["IMG"], cfg["BLK"]
    ND0, ND1 = cfg["ND0"], cfg["ND1"]
    SS, DS0, DS1 = cfg["SS"], cfg["DS0"], cfg["DS1"]
    S1, S2 = cfg["S1"], cfg["S2"]
    KI, KB, MB = IMG // 128, BLK // 128, F // 128
    O = F // H

    nc = bacc.Bacc("TRN2", target_bir_lowering=False, debug=False,
                   enable_asserts=True, num_devices=NCORE,
                   num_swdge_queues=4)

    def param(name, shape, dt):
        return nc.declare_dram_parameter(name, list(shape), dt, isOutput=False)

    imgT = param("imgT", [IMG, SS], bf16)
    blkT = param("blkT", [BLK, SS], bf16)
    wimgT = param("wimgT", [IMG, F], bf16)
    wblkT = param("wblkT", [BLK, F], bf16)
    wv = param("wv", [F, F], bf16)
    we = param("we", [F, F], bf16)
    fc1T = param("fc1T", [F, F], bf16)
    acat = param("acat", [F, 16], bf16)
    fc2T = param("fc2T", [F, OUT], bf16)
    a2s = param("a2s", [128, OUT], f32)
    a2d = param("a2d", [128, OUT], f32)
    biasv = param("biasv", [128, MB], f32)
    biase = param("biase", [128, MB], f32)
    iota = param("iota", [128, 128], f32)
    ident = param("ident", [128, 128], bf16)
    g1 = param("g1", [128, S1 // 16], i16)
    sd1 = param("sd1", [128, S1 // 16], i16)
    dc1 = param("dc1", [128, S1 // 128], f32)
    g2 = param("g2", [128, S2 // 16], i16)
    sd2 = param("sd2", [128, S2 // 16], i16)
    dc2 = param("dc2", [128, S2 // 128], f32)
    tick = param("tick", [128, 1], f32)
    out = nc.declare_dram_parameter("out", [DS1, OUT], f32, isOutput=True)
    tock = nc.declare_dram_parameter("tock", [128, 1], f32, isOutput=True)

    half1, half2 = cfg["half1"], cfg["half2"]
    table1 = nc.dram_tensor("table1", [SS, ROW1], bf16)
    sdsh1 = nc.dram_tensor("sdsh1", [SS, SROW], bf16)
    sdt1 = nc.dram_tensor("sdt1", [NS, SROW], bf16, addr_space="Shared")
    partials1a = nc.dram_tensor("partials1a", [NCORE * half1, PR1], bf16)
    partials1b = nc.dram_tensor("partials1b", [NCORE * half1, PR1], bf16)
    rs1a = nc.dram_tensor("rs1a", [half1, PR1], bf16)
    rs1b = nc.dram_tensor("rs1b", [half1, PR1], bf16)
    table2 = nc.dram_tensor("table2", [DS0, ROW2], bf16)
    sdsh2 = nc.dram_tensor("sdsh2", [DS0, SROW], bf16)
    sdt2 = nc.dram_tensor("sdt2", [ND0, SROW], bf16, addr_space="Shared")
    partials2a = nc.dram_tensor("partials2a", [NCORE * half2, PR2], bf16)
    partials2b = nc.dram_tensor("partials2b", [NCORE * half2, PR2], bf16)
    rs2a = nc.dram_tensor("rs2a", [half2, PR2], bf16)
    rs2b = nc.dram_tensor("rs2b", [half2, PR2], bf16)

    Sig = mybir.ActivationFunctionType.Sigmoid
    Exp = mybir.ActivationFunctionType.Exp
    TT = nc.vector.tensor_tensor
    MUL = mybir.AluOpType.mult
    ADD = mybir.AluOpType.add

    from contextlib import ExitStack
    if True:
      with tile.TileContext(nc) as tc, ExitStack() as top:
        res = top.enter_context(tc.tile_pool(name="res", bufs=1))
        # resident weights / constants
        wimg_sb = res.tile([128, KI * F], bf16)
        nc.sync.dma_start(wimg_sb[:].rearrange("p (k m) -> p k m", k=KI),
                          wimgT[:, :].rearrange("(k p) m -> p k m", p=128))
        wblk_sb = res.tile([128, KB * F], bf16)
        nc.sync.dma_start(wblk_sb[:].rearrange("p (k m) -> p k m", k=KB),
                          wblkT[:, :].rearrange("(k p) m -> p k m", p=128))
        wv_sb = res.tile([128, MB * F], bf16)
        nc.sync.dma_start(wv_sb[:].rearrange("p (k m) -> p k m", k=MB),
                          wv[:, :].rearrange("(k p) m -> p k m", p=128))
        we_sb = res.tile([128, MB * F], bf16)
        nc.sync.dma_start(we_sb[:].rearrange("p (k m) -> p k m", k=MB),
                          we[:, :].rearrange("(k p) m -> p k m", p=128))
        fc1_sb = res.tile([128, MB * F], bf16)
        nc.sync.dma_start(fc1_sb[:].rearrange("p (k m) -> p k m", k=MB),
                          fc1T[:, :].rearrange("(k p) m -> p k m", p=128))
        acat_sb = res.tile([128, MB * 16], bf16)
        nc.sync.dma_start(acat_sb[:].rearrange("p (k m) -> p k m", k=MB),
                          acat[:, :].rearrange("(k p) m -> p k m", p=128))
        fc2_sb = res.tile([128, MB * OUT], bf16)
        nc.sync.dma_start(fc2_sb[:].rearrange("p (k m) -> p k m", k=MB),
                          fc2T[:, :].rearrange("(k p) m -> p k m", p=128))
        a2s_sb = res.tile([128, OUT], f32)
        nc.sync.dma_start(a2s_sb[:], a2s[:, :])
        a2d_sb = res.tile([128, OUT], f32)
        nc.sync.dma_start(a2d_sb[:], a2d[:, :])
        bv_sb = res.tile([128, MB], f32)
        nc.sync.dma_start(bv_sb[:], biasv[:, :])
        be_sb = res.tile([128, MB], f32)
        nc.sync.dma_start(be_sb[:], biase[:, :])
        iota_sb = res.tile([128, 128], f32)
        nc.sync.dma_start(iota_sb[:], iota[:, :])
        id_sb = res.tile([128, 128], bf16)
        nc.sync.dma_start(id_sb[:], ident[:, :])
        g1_sb = res.tile([128, S1 // 16], i16)
        nc.sync.dma_start(g1_sb[:], g1[:, :])
        sd1_sb = res.tile([128, S1 // 16], i16)
        nc.sync.dma_start(sd1_sb[:], sd1[:, :])
        dc1_sb = res.tile([128, S1 // 128], f32)
        nc.sync.dma_start(dc1_sb[:], dc1[:, :])
        g2_sb = res.tile([128, S2 // 16], i16)
        nc.sync.dma_start(g2_sb[:], g2[:, :])
        sd2_sb = res.tile([128, S2 // 16], i16)
        nc.sync.dma_start(sd2_sb[:], sd2[:, :])
        dc2_sb = res.tile([128, S2 // 128], f32)
        nc.sync.dma_start(dc2_sb[:], dc2[:, :])

        # chain for timing
        tk = res.tile([128, 1], f32)
        nc.sync.dma_start(tk[:], tick[:, :])
        nc.sync.dma_start(tock[:, :], tk[:])
        if stop < 9:
            zo = res.tile([128, OUT], f32)
            nc.vector.memset(zo[:], 0.0)
            for tt in range(_ceil(DS1, 128)):
                rows = min(128, DS1 - tt * 128)
                nc.sync.dma_start(out[tt * 128:tt * 128 + rows, :],
                                  zo[:rows, :])

        for _rep in range(REPEAT):
          if _rep:
              # serialize repeats so repeat-K timing measures a full
              # dependency-honest iteration (idempotent reps would
              # otherwise overlap through untracked DRAM reuse)
              tc.strict_bb_all_engine_barrier()
          # ---------------- Phase A ----------------
          WA = min(500, SS)
          with ExitStack() as pa:
              rhsp = pa.enter_context(tc.tile_pool(name=f"parhs{_rep}", bufs=2))
              sbp = pa.enter_context(tc.tile_pool(name=f"pasb{_rep}", bufs=2))
              psp = pa.enter_context(tc.tile_pool(name=f"paps{_rep}", bufs=4, space="PSUM"))
              pst = pa.enter_context(tc.tile_pool(name=f"patr{_rep}", bufs=2, space="PSUM"))
              stp = pa.enter_context(tc.tile_pool(name=f"past{_rep}", bufs=3))
              for nt in range(_ceil(SS, WA)):
                  n0 = nt * WA
                  w = min(WA, SS - n0)
                  x_sb = rhsp.tile([128, KI * w], bf16, tag="x")
                  nc.sync.dma_start(
                      x_sb[:].rearrange("p (k n) -> p k n", k=KI),
                      imgT[:, n0:n0 + w].rearrange("(k p) n -> p k n", p=128))
                  b_sb = rhsp.tile([128, KB * w], bf16, tag="b")
                  nc.sync.dma_start(
                      b_sb[:].rearrange("p (k n) -> p k n", k=KB),
                      blkT[:, n0:n0 + w].rearrange("(k p) n -> p k n", p=128))

                  def mm(lhs_sb, rhs_sb, K, m, width):
                      ps = psp.tile([128, width], f32, tag="ps")
                      for k in range(K):
                          nc.tensor.matmul(
                              ps[:],
                              lhs_sb[:, (k * F + m * 128):(k * F + m * 128) + 128],
                              rhs_sb[:, k * width:(k + 1) * width],
                              start=(k == 0), stop=(k == K - 1))
                      return ps

                  fi_sb = sbp.tile([128, MB * w], bf16, tag="fi")
                  ti_sb = sbp.tile([128, MB * w], bf16, tag="ti")
                  av_sb = sbp.tile([128, MB * w], bf16, tag="av")
                  ae_sb = sbp.tile([128, MB * w], bf16, tag="ae")
                  for m in range(MB):
                      ps = mm(wimg_sb, x_sb, KI, m, w)
                      nc.vector.tensor_copy(fi_sb[:, m * w:(m + 1) * w], ps[:])
                  for m in range(MB):
                      ps = mm(wblk_sb, b_sb, KB, m, w)
                      nc.vector.tensor_copy(ti_sb[:, m * w:(m + 1) * w], ps[:])
                  for m in range(MB):
                      ps = mm(wv_sb, fi_sb, MB, m, w)
                      nc.scalar.activation(av_sb[:, m * w:(m + 1) * w], ps[:],
                                           Sig, bias=bv_sb[:, m:m + 1])
                  for m in range(MB):
                      ps = mm(we_sb, ti_sb, MB, m, w)
                      nc.scalar.activation(ae_sb[:, m * w:(m + 1) * w], ps[:],
                                           Sig, bias=be_sb[:, m:m + 1])
                  fu_sb = sbp.tile([128, MB * w], bf16, tag="fu")
                  TT(fu_sb[:], av_sb[:], fi_sb[:], MUL)
                  TT(ae_sb[:], ae_sb[:], ti_sb[:], MUL)
                  TT(fu_sb[:], fu_sb[:], ae_sb[:], ADD)
                  z_sb = sbp.tile([128, MB * w], bf16, tag="z")
                  for m in range(MB):
                      ps = mm(fc1_sb, fu_sb, MB, m, w)
                      nc.vector.tensor_copy(z_sb[:, m * w:(m + 1) * w], ps[:])
                  pss = psp.tile([128, w], f32, tag="ps")
                  for k in range(MB):
                      nc.tensor.matmul(pss[:16, :], acat_sb[:, k * 16:(k + 1) * 16],
                                       fu_sb[:, k * w:(k + 1) * w],
                                       start=(k == 0), stop=(k == MB - 1))
                  s_sb = sbp.tile([16, w], bf16, tag="s")
                  nc.vector.tensor_copy(s_sb[:], pss[:16, :])
                  for b0 in range(0, w, 128):
                      wb = min(128, w - b0)
                      st = stp.tile([128, ROW1], bf16, tag="t1")
                      nc.vector.memset(st[:, F + 16:ROW1], 0.0)
                      for m in range(MB):
                          ptr = pst.tile([128, 128], bf16, tag="tr")
                          nc.tensor.matmul(ptr[:wb, :],
                                           z_sb[:, m * w + b0:m * w + b0 + wb],
                                           id_sb[:], is_transpose=True)
                          nc.vector.tensor_copy(
                              st[:wb, m * 128:(m + 1) * 128], ptr[:wb, :])
                      ptr = pst.tile([128, 128], bf16, tag="tr")
                      nc.tensor.matmul(ptr[:wb, :16], s_sb[:, b0:b0 + wb],
                                       id_sb[:16, :16], is_transpose=True)
                      nc.vector.tensor_copy(st[:wb, F:F + 16], ptr[:wb, :16])
                      nc.sync.dma_start(table1[n0 + b0:n0 + b0 + wb, :],
                                        st[:wb, :])
                      st2 = stp.tile([128, SROW], bf16, tag="t2")
                      nc.vector.memset(st2[:, 8:SROW], 0.0)
                      nc.vector.tensor_copy(st2[:wb, 0:8], st[:wb, F + 8:F + 16])
                      nc.sync.dma_start(sdsh1[n0 + b0:n0 + b0 + wb, :],
                                        st2[:wb, :])

          if stop >= 2:
              nc.gpsimd.collective_compute(
                  "AllGather", mybir.AluOpType.bypass,
                  replica_groups=[list(range(NCORE))],
                  ins=[sdsh1[:, :]], outs=[sdt1[:, :]])

          # ---------------- Layer-1 aggregation ----------------
          def _rs1a():
              nc.gpsimd.collective_compute(
                  "ReduceScatter", ADD, replica_groups=[list(range(NCORE))],
                  ins=[partials1a[:, :]], outs=[rs1a[:, :]])

          with ExitStack() as ag1:
            if stop >= 3:
              _agg_layer(nc, tc, ag1, cfg, table=table1, sdt=sdt1,
                         gsb=g1_sb, sdsb=sd1_sb, dcsb=dc1_sb, iota_sb=iota_sb,
                         calls=cfg["calls1"], cm=cfg["cm1"],
                         row=ROW1, srow=SROW, prow=PR1, zw=F, nh=H,
                         pa=partials1a, pb=partials1b, routing=cfg["rt1"],
                         hooks=({cfg["sp1"]: _rs1a} if stop >= 4 else None),
                         rep=_rep)

          if stop >= 4:
              nc.gpsimd.collective_compute(
                  "ReduceScatter", ADD, replica_groups=[list(range(NCORE))],
                  ins=[partials1b[:, :]], outs=[rs1b[:, :]])

          # ---------------- normalize + layer-2 tables ----------------
          with ExitStack() as p4:
            if stop >= 5:
              sbp = p4.enter_context(tc.tile_pool(name=f"n2sb{_rep}", bufs=3))
              psp = p4.enter_context(tc.tile_pool(name=f"n2ps{_rep}", bufs=2, space="PSUM"))
              ptp = p4.enter_context(tc.tile_pool(name=f"n2pt{_rep}", bufs=2, space="PSUM"))
              for tt in range(_ceil(DS0, 128)):
                  r0 = tt * 128
                  rows = min(128, DS0 - r0)
                  hs = sbp.tile([128, PR1], bf16, tag="hs")
                  for (h_sel, h_lo, h_ln, h_dst) in _split_rows(
                          r0, rows, half1):
                      nc.sync.dma_start(
                          hs[h_lo:h_lo + h_ln, :],
                          (rs1a if h_sel == 0 else rs1b)[
                              h_dst:h_dst + h_ln, :])
                  rden = sbp.tile([128, H], f32, tag="rd")
                  nc.vector.reciprocal(rden[:rows, :], hs[:rows, F:F + H])
                  hraw = sbp.tile([128, F], f32, tag="hraw")
                  TT(hraw[:rows, :].rearrange("p (h o) -> p h o", h=H),
                     hs[:rows, 0:F].rearrange("p (h o) -> p h o", h=H),
                     rden[:rows, :].unsqueeze(2).broadcast_to(
                         [rows, H, F // H]),
                     MUL)
                  t1 = sbp.tile([128, F], f32, tag="t1")
                  nc.vector.tensor_scalar_min(t1[:rows, :], hraw[:rows, :], 0.0)
                  nc.scalar.activation(t1[:rows, :], t1[:rows, :], Exp)
                  h1 = sbp.tile([128, F], bf16, tag="h1")
                  nc.vector.scalar_tensor_tensor(
                      h1[:rows, :], t1[:rows, :], -1.0, hraw[:rows, :],
                      ADD, mybir.AluOpType.max)
                  h1t = sbp.tile([128, MB * 128], bf16, tag="h1t")
                  for m in range(MB):
                      ptr = ptp.tile([128, 128], bf16, tag="tr")
                      nc.tensor.matmul(ptr[:, :rows],
                                       h1[:rows, m * 128:(m + 1) * 128],
                                       id_sb[:rows, :rows], is_transpose=True)
                      nc.vector.tensor_copy(h1t[:, m * 128:m * 128 + rows],
                                            ptr[:, :rows])
                  pz2 = psp.tile([128, OUT], f32, tag="z2")
                  for k in range(MB):
                      nc.tensor.matmul(pz2[:rows, :],
                                       h1t[:, k * 128:k * 128 + rows],
                                       fc2_sb[:, k * OUT:(k + 1) * OUT],
                                       start=(k == 0), stop=(k == MB - 1))
                  scr = sbp.tile([128, OUT], f32, tag="scr")
                  s2s = sbp.tile([128, 1], f32, tag="s2s")
                  s2d = sbp.tile([128, 1], f32, tag="s2d")
                  if P4_LEVEL >= 3:
                      nc.vector.tensor_tensor_reduce(
                          scr[:rows, :], pz2[:rows, :], a2s_sb[:rows, :], 1.0,
                          0.0, MUL, ADD, s2s[:rows, :])
                      nc.vector.tensor_tensor_reduce(
                          scr[:rows, :], pz2[:rows, :], a2d_sb[:rows, :], 1.0,
                          0.0, MUL, ADD, s2d[:rows, :])
                  else:
                      nc.vector.tensor_tensor(scr[:rows, :], pz2[:rows, :],
                                              a2s_sb[:rows, :], MUL)
                      nc.vector.reduce_sum(s2s[:rows, :], scr[:rows, :],
                                           mybir.AxisListType.X)
                      nc.vector.tensor_tensor(scr[:rows, :], pz2[:rows, :],
                                              a2d_sb[:rows, :], MUL)
                      nc.vector.reduce_sum(s2d[:rows, :], scr[:rows, :],
                                           mybir.AxisListType.X)
                  st = sbp.tile([128, ROW2], bf16, tag="st")
                  nc.vector.memset(st[:, OUT + 1:ROW2], 0.0)
                  nc.vector.tensor_copy(st[:rows, 0:OUT], pz2[:rows, :])
                  nc.vector.tensor_copy(st[:rows, OUT:OUT + 1], s2s[:rows, :])
                  nc.sync.dma_start(table2[r0:r0 + rows, :], st[:rows, :])
                  st2 = sbp.tile([128, SROW], bf16, tag="st2")
                  nc.vector.memset(st2[:, 1:SROW], 0.0)
                  nc.vector.tensor_copy(st2[:rows, 0:1], s2d[:rows, :])
                  nc.sync.dma_start(sdsh2[r0:r0 + rows, :], st2[:rows, :])

          if stop >= 6:
              nc.gpsimd.collective_compute(
                  "AllGather", mybir.AluOpType.bypass,
                  replica_groups=[list(range(NCORE))],
                  ins=[sdsh2[:, :]], outs=[sdt2[:, :]])

          # ---------------- Layer-2 aggregation ----------------
          def _rs2a():
              nc.gpsimd.collective_compute(
                  "ReduceScatter", ADD, replica_groups=[list(range(NCORE))],
                  ins=[partials2a[:, :]], outs=[rs2a[:, :]])

          with ExitStack() as ag2:
            if stop >= 7:
              _agg_layer(nc, tc, ag2, cfg, table=table2, sdt=sdt2,
                         gsb=g2_sb, sdsb=sd2_sb, dcsb=dc2_sb, iota_sb=iota_sb,
                         calls=cfg["calls2"], cm=cfg["cm2"],
                         row=ROW2, srow=SROW, prow=PR2, zw=OUT, nh=1,
                         pa=partials2a, pb=partials2b, routing=cfg["rt2"],
                         hooks=({cfg["sp2"]: _rs2a} if stop >= 8 else None),
                         rep=_rep)

          if stop >= 8:
              nc.gpsimd.collective_compute(
                  "ReduceScatter", ADD, replica_groups=[list(range(NCORE))],
                  ins=[partials2b[:, :]], outs=[rs2b[:, :]])

          # ---------------- final normalize ----------------
          with ExitStack() as p8:
            if stop >= 9:
              sbp = p8.enter_context(tc.tile_pool(name=f"fsb{_rep}", bufs=3))
              for tt in range(_ceil(DS1, 128)):
                  r0 = tt * 128
                  rows = min(128, DS1 - r0)
                  hs = sbp.tile([128, PR2], bf16, tag="hs")
                  for (h_sel, h_lo, h_ln, h_dst) in _split_rows(
                          r0, rows, half2):
                      nc.sync.dma_start(
                          hs[h_lo:h_lo + h_ln, :],
                          (rs2a if h_sel == 0 else rs2b)[
                              h_dst:h_dst + h_ln, :])
                  rden = sbp.tile([128, 1], f32, tag="rd")
                  nc.vector.reciprocal(rden[:rows, :], hs[:rows, OUT:OUT + 1])
                  ot = sbp.tile([128, OUT], f32, tag="ot")
                  TT(ot[:rows, :], hs[:rows, 0:OUT],
                     rden[:rows, :].broadcast_to([rows, OUT]), MUL)
                  nc.sync.dma_start(out[r0:r0 + rows, :], ot[:rows, :])

    nc.compile()
    return nc


_CACHE = {}


def _get_nc(cfg):
    key = repr(sorted((k, v) for k, v in cfg.items()))
    if key not in _CACHE:
        _CACHE[key] = _build(cfg)
    return _CACHE[key]


def kernel(**inputs) -> np.ndarray:
    cfg, in_maps = _prep(inputs)
    nc = _get_nc(cfg)
    res = run_bass_kernel_spmd(nc, in_maps, core_ids=list(range(NCORE)))
    return np.concatenate([res.results[c]["out"] for c in range(NCORE)],
                          axis=0)



# revision 17
# speedup vs baseline: 1.1605x; 1.0275x over previous
"""Trainium2 Bass kernel for a 2-layer GAT (cross-attention fusion + 8-head GAT
+ 1-head GAT) distributed over 8 NeuronCores.

Strategy (src-sharded message passing, all gathers local):
  - Phase A: per-node feature transforms sharded by src node (NS/8 per core),
    activations kept transposed [feat, node] so matmuls contract on partitions.
    Each core writes a local gather-table row per owned node:
      [z(512) | s_src(8) | s_dst(8) | pad] bf16, 1280B rows.
  - A small AllGather replicates a padded per-node s_dst table (256B rows).
  - Layer-1 aggregation: each core processes the edges whose src it owns.
    dma_gather fetches z rows from the LOCAL table (int16 idx ok), a second
    gather fetches s_dst by global dst. Per-edge exp(leakyrelu(s_src+s_dst))
    weights are applied, and one-hot matmuls accumulate exp-weighted partial
    sums + denominators per global dst tile in PSUM.
  - One bf16 ReduceScatter sums partials [ND0, 520] across cores; each core
    ends with its own dst range = its layer-2 src shard.
  - Normalize + ELU + z2 projection per owned node -> local layer-2 table,
    then the same aggregation scheme for layer 2 and a final ReduceScatter.
"""
import os
import sys
import math

sys.path.insert(0, "/opt/trn_rl_repo")

import numpy as np
import ml_dtypes

import concourse.bass as bass
import concourse.bacc as bacc
import concourse.tile as tile
import concourse.mybir as mybir
from concourse.bass_utils import run_bass_kernel_spmd

BF16 = ml_dtypes.bfloat16
NCORE = 8
F = 512          # fused dim
H = 8            # layer-1 heads
OUT = 128        # layer-2 out dim
ROW1 = 640       # layer-1 table row, bf16 elems (1280B): [z 512|s_src 8|s_dst 8|pad]
SROW = 128       # s_dst table row, bf16 elems (256B)
ROW2 = 256       # layer-2 table row (512B): [z2 128|s2src 1|pad]
PR1 = 520        # layer-1 partial row: [h 512 | den 8]
PR2 = 129        # layer-2 partial row: [h 128 | den 1]
BUDGET_CHUNKS = 3    # chunks per dma_gather call (4 SWDGE queues in flight)


def _ceil(a, b):
    return -(-a // b)


def _wrap_idx(idx):
    """[S] -> [128, S//16] int16, wrapped in 16 partitions, replicated 8x."""
    w = idx.reshape(-1, 16).T.astype(np.int16)
    return np.ascontiguousarray(np.tile(w, (8, 1)))


def _routing(t, n_dst, own, half):
    """Store routing for dst tile t: list of (sel, src_lo, ln, dst_lo).

    Global partial rows are re-laid-out as two half tensors so a
    ReduceScatter over each delivers core c exactly the first/second half
    of its owned range: half-0 rows = concat_c [own*c, own*c+half),
    half-1 rows = concat_c [own*c+half, own*(c+1))."""
    r0, r1 = 128 * t, min(128 * (t + 1), n_dst)
    out = []
    r = r0
    while r < r1:
        c, loc = r // own, r % own
        sel = 0 if loc < half else 1
        seg_end = own * c + (half if sel == 0 else own)
        ln = min(r1, seg_end) - r
        dst = half * c + (loc if sel == 0 else loc - half)
        out.append((sel, r - r0, ln, dst))
        r += ln
    return tuple(out)


def _sched(src, dst, shard, n_dst, own):
    """Static per-dst-tile schedule, uniform across cores.

    Tiles are processed group-A-first (tiles containing any first-half
    row of a core's owned range) so the half-0 partials tensor is
    complete mid-aggregation and its ReduceScatter overlaps the rest.

    Returns (chunkmeta, calls, S, g[8,S], sd[8,S], dcol[8,S],
             routing, split_call, half)."""
    T = _ceil(n_dst, 128)
    half = own // 2
    owner = src // shard
    percore = []
    cnt = np.zeros((NCORE, T), np.int64)
    for c in range(NCORE):
        m = owner == c
        s_loc = (src[m] - c * shard).astype(np.int64)
        d = dst[m].astype(np.int64)
        o = np.argsort(d, kind="stable")
        s_loc, d = s_loc[o], d[o]
        t = d // 128
        cnt[c] = np.bincount(t, minlength=T)
        percore.append((s_loc, d, t))
    C = np.maximum(1, _ceil_arr(cnt.max(axis=0), 128))
    routing = {t: _routing(t, n_dst, own, half) for t in range(T)}
    grpA = [t for t in range(T) if any(s == 0 for s, _, _, _ in routing[t])]
    grpB = [t for t in range(T) if t not in set(grpA)]
    perm = grpA + grpB
    chunkmeta = []
    tile_base = np.zeros(T, np.int64)
    off = 0
    nchunks_A = 0
    for t in perm:
        tile_base[t] = off
        for p in range(int(C[t])):
            chunkmeta.append((t, p == 0, p == C[t] - 1))
        off += C[t] * 128
        if t in set(grpA):
            nchunks_A += int(C[t])
    S = int(off)
    calls = []
    cur_off = cur_n = 0
    for t in perm:
        if cur_n + C[t] > BUDGET_CHUNKS and cur_n > 0:
            calls.append((cur_off, cur_n))
            cur_off += cur_n
            cur_n = 0
        cur_n += int(C[t])
    calls.append((cur_off, cur_n))
    # call whose chunks complete group A -> hook point for the first RS
    split_call = 0
    acc = 0
    for i, (coff, n) in enumerate(calls):
        acc += n
        if acc >= nchunks_A:
            split_call = i
            break
    g = np.zeros((NCORE, S), np.int64)
    sd = np.zeros((NCORE, S), np.int64)
    dcol = np.full((NCORE, S), -1.0, np.float32)
    for c in range(NCORE):
        s_loc, d, t = percore[c]
        starts = np.searchsorted(t, np.arange(T))
        pos = np.arange(len(t)) - starts[t]
        slot = tile_base[t] + pos
        g[c, slot] = s_loc
        sd[c, slot] = d
        dcol[c, slot] = (d - t * 128).astype(np.float32)
    routing_t = tuple(routing[t] for t in range(T))
    return chunkmeta, calls, S, g, sd, dcol, routing_t, split_call, half


def _ceil_arr(a, b):
    return -(-a // b)


def _prep(inputs):
    img = np.asarray(inputs["image_features"], np.float32)
    blk = np.asarray(inputs["block_features"], np.float32)
    W_img = np.asarray(inputs["W_img"], np.float32)
    W_blk = np.asarray(inputs["W_blk"], np.float32)
    Wv = np.asarray(inputs["Wv"], np.float32)
    bv = np.asarray(inputs["bv"], np.float32)
    We = np.asarray(inputs["We"], np.float32)
    be = np.asarray(inputs["be"], np.float32)
    fc1 = np.asarray(inputs["fc1"], np.float32)
    attn1 = np.asarray(inputs["attn1"], np.float32)
    fc2 = np.asarray(inputs["fc2"], np.float32)
    attn2 = np.asarray(inputs["attn2"], np.float32)
    e0s = np.asarray(inputs["edge0_src"], np.int64)
    e0d = np.asarray(inputs["edge0_dst"], np.int64)
    e1s = np.asarray(inputs["edge1_src"], np.int64)
    e1d = np.asarray(inputs["edge1_dst"], np.int64)
    ND0 = int(inputs["n_dst0"])
    ND1 = int(inputs["n_dst1"])

    NS, IMG = img.shape
    BLK = blk.shape[1]
    assert W_img.shape == (F, IMG) and W_blk.shape == (F, BLK)
    assert fc1.shape == (H, F // H, F) and fc2.shape[1] == OUT
    assert NS % NCORE == 0 and ND0 % NCORE == 0 and ND1 % NCORE == 0
    assert IMG % 128 == 0 and BLK % 128 == 0
    SS, DS0, DS1 = NS // NCORE, ND0 // NCORE, ND1 // NCORE

    O = F // H
    # host-derived weights
    wimgT = np.ascontiguousarray(W_img.T).astype(BF16)          # [IMG, F]
    wblkT = np.ascontiguousarray(W_blk.T).astype(BF16)          # [BLK, F]
    wv = Wv.astype(BF16)                                        # [F, F] lhsT
    we = We.astype(BF16)
    fc1T = np.ascontiguousarray(fc1.reshape(F, F).T).astype(BF16)   # [F, (h o)]
    a_src = np.einsum("hof,ho->fh", fc1, attn1[:, :O])
    a_dst = np.einsum("hof,ho->fh", fc1, attn1[:, O:])
    acat = np.concatenate([a_src, a_dst], axis=1).astype(BF16)  # [F, 16]
    fc2T = np.ascontiguousarray(fc2[0].T).astype(BF16)          # [F, OUT]
    a2s = np.tile(attn2[0, :OUT].astype(np.float32), (128, 1))  # [128, OUT]
    a2d = np.tile(attn2[0, OUT:].astype(np.float32), (128, 1))
    MB = F // 128
    biasv = np.ascontiguousarray(bv.reshape(MB, 128).T).astype(np.float32)
    biase = np.ascontiguousarray(be.reshape(MB, 128).T).astype(np.float32)
    iota = np.tile(np.arange(128, dtype=np.float32), (128, 1))
    ident = np.eye(128, dtype=np.float32).astype(BF16)

    (cm1, calls1, S1, g1, sd1, dc1,
     rt1, sp1, half1) = _sched(e0s, e0d, SS, ND0, ND0 // NCORE)
    (cm2, calls2, S2, g2, sd2, dc2,
     rt2, sp2, half2) = _sched(e1s, e1d, DS0, ND1, ND1 // NCORE)

    shared = dict(wimgT=wimgT, wblkT=wblkT, wv=wv, we=we, fc1T=fc1T, acat=acat,
                  fc2T=fc2T, a2s=a2s, a2d=a2d, biasv=biasv, biase=biase,
                  iota=iota, ident=ident,
                  tick=np.zeros((128, 1), np.float32))
    in_maps = []
    for c in range(NCORE):
        m = dict(shared)
        m["imgT"] = np.ascontiguousarray(
            img[c * SS:(c + 1) * SS].T).astype(BF16)
        m["blkT"] = np.ascontiguousarray(
            blk[c * SS:(c + 1) * SS].T).astype(BF16)
        m["g1"] = _wrap_idx(g1[c])
        m["sd1"] = _wrap_idx(sd1[c])
        m["dc1"] = np.ascontiguousarray(dc1[c].reshape(-1, 128).T)
        m["g2"] = _wrap_idx(g2[c])
        m["sd2"] = _wrap_idx(sd2[c])
        m["dc2"] = np.ascontiguousarray(dc2[c].reshape(-1, 128).T)
        in_maps.append(m)

    cfg = dict(NS=NS, IMG=IMG, BLK=BLK, ND0=ND0, ND1=ND1,
               SS=SS, DS0=DS0, DS1=DS1,
               cm1=tuple(cm1), calls1=tuple(calls1), S1=S1,
               cm2=tuple(cm2), calls2=tuple(calls2), S2=S2,
               rt1=rt1, sp1=sp1, half1=half1,
               rt2=rt2, sp2=sp2, half2=half2)
    return cfg, in_maps


# ---------------------------------------------------------------- device code

AGG_LEVEL = int(os.environ.get("GAT_AGG_LEVEL", "3"))
P4_LEVEL = int(os.environ.get("GAT_P4_LEVEL", "2"))


def _agg_layer(nc, tc, ctx, cfg, *, table, sdt, gsb, sdsb, dcsb, iota_sb,
               calls, cm, row, srow, prow, zw, nh, pa, pb, routing,
               hooks=None, rep=0):
    """Shared edge-aggregation pipeline for both GAT layers.

    row: gather row width (elems); srow: s_dst row width; prow: partial row;
    zw: z width (512 or 128); nh: heads (8 or 1).
    Layer-1 rows: [z 512 | s_src 8 | s_dst 8 | pad]; s-gather gives s_dst.
    Layer-2 rows: [z2 128 | s2src 1 | pad]; s-gather gives s2dst.
    """
    bf16 = mybir.dt.bfloat16
    f32 = mybir.dt.float32
    maxch = max(n for _, n in calls)
    gb = ctx.enter_context(tc.tile_pool(name=f"gb{zw}_{rep}", bufs=6))
    sdb = ctx.enter_context(tc.tile_pool(name=f"sdb{zw}_{rep}", bufs=6))
    ohb = ctx.enter_context(tc.tile_pool(name=f"ohb{zw}_{rep}", bufs=3))
    zsb = ctx.enter_context(tc.tile_pool(name=f"zsb{zw}_{rep}", bufs=3))
    escb = ctx.enter_context(tc.tile_pool(name=f"escb{zw}_{rep}", bufs=3))
    stg = ctx.enter_context(tc.tile_pool(name=f"stg{zw}_{rep}", bufs=4))
    ph = ctx.enter_context(tc.tile_pool(name=f"ph{zw}_{rep}", bufs=2, space="PSUM"))
    pd = ctx.enter_context(tc.tile_pool(name=f"pd{zw}_{rep}", bufs=2, space="PSUM"))

    zcol = zw + nh  # matmul rhs width: [z | exp]
    Exp = mybir.ActivationFunctionType.Exp
    cur_ph = cur_pd = None
    for ci, (coff, nch) in enumerate(calls):
        soff = coff * 128
        nidx = nch * 128
        gt = gb.tile([128, nch * row], bf16, tag="gt")
        nc.gpsimd.dma_gather(
            gt[:].rearrange("p (c e) -> p c e", e=row),
            table[:, :], gsb[:, soff // 16:(soff + nidx) // 16],
            nidx, nidx, row, queue_num=ci % 4)
        sdt_t = sdb.tile([128, nch * srow], bf16, tag="sdt")
        nc.gpsimd.dma_gather(
            sdt_t[:].rearrange("p (c e) -> p c e", e=srow),
            sdt[:, :], sdsb[:, soff // 16:(soff + nidx) // 16],
            nidx, nidx, srow, queue_num=(ci + 2) % 4)
        g3 = gt[:].rearrange("p (c e) -> p c e", e=row)
        s3 = sdt_t[:].rearrange("p (c e) -> p c e", e=srow)
        if AGG_LEVEL <= 1:
            st = stg.tile([128, prow], bf16, tag="st")
            nc.vector.tensor_copy(st[:, 0:prow], gt[:, 0:prow])
            nc.vector.tensor_copy(st[:, 0:srow // 2], sdt_t[:, 0:srow // 2])
            nc.sync.dma_start(pa[0:128, 0:prow], st[:, 0:prow])
            continue
        # one-hot [128e, nch, 128d]
        oh = ohb.tile([128, nch * 128], bf16, tag="oh")
        nc.vector.tensor_tensor(
            oh[:].rearrange("p (c d) -> p c d", d=128),
            iota_sb[:].unsqueeze(1).broadcast_to([128, nch, 128]),
            dcsb[:, coff:coff + nch].unsqueeze(2).broadcast_to([128, nch, 128]),
            mybir.AluOpType.is_equal)
        # escore = s_src (gathered row) + s_dst (s-gather)
        esc = escb.tile([128, nch * nh], f32, tag="esc")
        e3 = esc[:].rearrange("p (c h) -> p c h", h=nh)
        nc.vector.tensor_tensor(e3, g3[:, :, zw:zw + nh], s3[:, :, 0:nh],
                                mybir.AluOpType.add)
        nc.vector.scalar_tensor_tensor(esc[:], esc[:], 0.01, esc[:],
                                       mybir.AluOpType.mult,
                                       mybir.AluOpType.max)
        zs = zsb.tile([128, nch * zcol], bf16, tag="zs")
        z3 = zs[:].rearrange("p (c e) -> p c e", e=zcol)
        nc.scalar.activation(z3[:, :, zw:zcol], e3, Exp)
        # z * exp broadcast per head
        nc.vector.tensor_tensor(
            z3[:, :, 0:zw].rearrange("p c (h o) -> p c h o", h=nh),
            g3[:, :, 0:zw].rearrange("p c (h o) -> p c h o", h=nh),
            z3[:, :, zw:zcol].unsqueeze(3).broadcast_to(
                [128, nch, nh, zw // nh]),
            mybir.AluOpType.mult)
        if zw + nh <= 512:
            # single fused matmul per chunk: rhs [z | exp]
            for j in range(nch):
                t, first, last = cm[coff + j]
                if first:
                    cur_ph = ph.tile([128, zw + nh], f32, tag="ph")
                nc.tensor.matmul(cur_ph[:], oh[:, j * 128:(j + 1) * 128],
                                 zs[:, j * zcol:(j + 1) * zcol],
                                 start=first, stop=last)
                if last:
                    st = stg.tile([128, prow], bf16, tag="st")
                    nc.vector.tensor_copy(st[:, 0:zw + nh], cur_ph[:])
                    for sel, src_lo, ln, dst_lo in routing[t]:
                        nc.sync.dma_start(
                            (pa if sel == 0 else pb)[dst_lo:dst_lo + ln,
                                                     0:zw + nh],
                            st[src_lo:src_lo + ln, 0:zw + nh])
        else:
            # two contiguous accumulation passes per tile (h then den):
            # interleaving open PSUM groups on PE crashes HW.
            jt = 0
            while jt < nch:
                t = cm[coff + jt][0]
                span = 0
                while jt + span < nch and cm[coff + jt + span][0] == t:
                    span += 1
                cur_ph = ph.tile([128, zw], f32, tag="ph")
                cur_pd = pd.tile([128, nh], f32, tag="pd")
                for q in range(span):
                    j = jt + q
                    nc.tensor.matmul(cur_ph[:], oh[:, j * 128:(j + 1) * 128],
                                     zs[:, j * zcol:j * zcol + zw],
                                     start=(q == 0), stop=(q == span - 1))
                for q in range(span):
                    j = jt + q
                    nc.tensor.matmul(cur_pd[:], oh[:, j * 128:(j + 1) * 128],
                                     zs[:, j * zcol + zw:(j + 1) * zcol],
                                     start=(q == 0), stop=(q == span - 1))
                st = stg.tile([128, prow], bf16, tag="st")
                nc.vector.tensor_copy(st[:, 0:zw], cur_ph[:])
                nc.vector.tensor_copy(st[:, zw:zw + nh], cur_pd[:])
                for sel, src_lo, ln, dst_lo in routing[t]:
                    nc.sync.dma_start(
                        (pa if sel == 0 else pb)[dst_lo:dst_lo + ln,
                                                 0:zw + nh],
                        st[src_lo:src_lo + ln, 0:zw + nh])
                jt += span
        if hooks and ci in hooks:
            hooks[ci]()


STOP_STAGE = int(os.environ.get("GAT_STOP_STAGE", "9"))


def _split_rows(r0, rows, half):
    """Split local row range [r0, r0+rows) at the half boundary.

    Yields (sel, lo_in_tile, ln, offset_in_half_tensor)."""
    out = []
    r = r0
    while r < r0 + rows:
        sel = 0 if r < half else 1
        end = min(r0 + rows, half if sel == 0 else r0 + rows)
        ln = end - r
        out.append((sel, r - r0, ln, r if sel == 0 else r - half))
        r += ln
    return out



def _build(cfg):
    stop = STOP_STAGE
    REPEAT = cfg.get("repeat", 1)
    bf16 = mybir.dt.bfloat16
    f32 = mybir.dt.float32
    i16 = mybir.dt.int16
    NS, IMG, BLK = cfg["NS"], cfg["IMG"], cfg["BLK"]
    ND0, ND1 = cfg["ND0"], cfg["ND1"]
    SS, DS0, DS1 = cfg["SS"], cfg["DS0"], cfg["DS1"]
    S1, S2 = cfg["S1"], cfg["S2"]
    KI, KB, MB = IMG // 128, BLK // 128, F // 128
    O = F // H

    nc = bacc.Bacc("TRN2", target_bir_lowering=False, debug=False,
                   enable_asserts=True, num_devices=NCORE,
                   num_swdge_queues=4)

    def param(name, shape, dt):
        return nc.declare_dram_parameter(name, list(shape), dt, isOutput=False)

    imgT = param("imgT", [IMG, SS], bf16)
    blkT = param("blkT", [BLK, SS], bf16)
    wimgT = param("wimgT", [IMG, F], bf16)
    wblkT = param("wblkT", [BLK, F], bf16)
    wv = param("wv", [F, F], bf16)
    we = param("we", [F, F], bf16)
    fc1T = param("fc1T", [F, F], bf16)
    acat = param("acat", [F, 16], bf16)
    fc2T = param("fc2T", [F, OUT], bf16)
    a2s = param("a2s", [128, OUT], f32)
    a2d = param("a2d", [128, OUT], f32)
    biasv = param("biasv", [128, MB], f32)
    biase = param("biase", [128, MB], f32)
    iota = param("iota", [128, 128], f32)
    ident = param("ident", [128, 128], bf16)
    g1 = param("g1", [128, S1 // 16], i16)
    sd1 = param("sd1", [128, S1 // 16], i16)
    dc1 = param("dc1", [128, S1 // 128], f32)
    g2 = param("g2", [128, S2 // 16], i16)
    sd2 = param("sd2", [128, S2 // 16], i16)
    dc2 = param("dc2", [128, S2 // 128], f32)
    tick = param("tick", [128, 1], f32)
    out = nc.declare_dram_parameter("out", [DS1, OUT], f32, isOutput=True)
    tock = nc.declare_dram_parameter("tock", [128, 1], f32, isOutput=True)

    half1, half2 = cfg["half1"], cfg["half2"]
    table1 = nc.dram_tensor("table1", [SS, ROW1], bf16)
    sdsh1 = nc.dram_tensor("sdsh1", [SS, SROW], bf16)
    sdt1 = nc.dram_tensor("sdt1", [NS, SROW], bf16, addr_space="Shared")
    partials1a = nc.dram_tensor("partials1a", [NCORE * half1, PR1], bf16)
    partials1b = nc.dram_tensor("partials1b", [NCORE * half1, PR1], bf16)
    rs1a = nc.dram_tensor("rs1a", [half1, PR1], bf16)
    rs1b = nc.dram_tensor("rs1b", [half1, PR1], bf16)
    table2 = nc.dram_tensor("table2", [DS0, ROW2], bf16)
    sdsh2 = nc.dram_tensor("sdsh2", [DS0, SROW], bf16)
    sdt2 = nc.dram_tensor("sdt2", [ND0, SROW], bf16, addr_space="Shared")
    partials2a = nc.dram_tensor("partials2a", [NCORE * half2, PR2], bf16)
    partials2b = nc.dram_tensor("partials2b", [NCORE * half2, PR2], bf16)
    rs2a = nc.dram_tensor("rs2a", [half2, PR2], bf16)
    rs2b = nc.dram_tensor("rs2b", [half2, PR2], bf16)

    Sig = mybir.ActivationFunctionType.Sigmoid
    Exp = mybir.ActivationFunctionType.Exp
    TT = nc.vector.tensor_tensor
    MUL = mybir.AluOpType.mult
    ADD = mybir.AluOpType.add

    from contextlib import ExitStack
    if True:
      with tile.TileContext(nc) as tc, ExitStack() as top:
        res = top.enter_context(tc.tile_pool(name="res", bufs=1))
        # resident weights / constants
        wimg_sb = res.tile([128, KI * F], bf16)
        nc.sync.dma_start(wimg_sb[:].rearrange("p (k m) -> p k m", k=KI),
                          wimgT[:, :].rearrange("(k p) m -> p k m", p=128))
        wblk_sb = res.tile([128, KB * F], bf16)
        nc.sync.dma_start(wblk_sb[:].rearrange("p (k m) -> p k m", k=KB),
                          wblkT[:, :].rearrange("(k p) m -> p k m", p=128))
        wv_sb = res.tile([128, MB * F], bf16)
        nc.sync.dma_start(wv_sb[:].rearrange("p (k m) -> p k m", k=MB),
                          wv[:, :].rearrange("(k p) m -> p k m", p=128))
        we_sb = res.tile([128, MB * F], bf16)
        nc.sync.dma_start(we_sb[:].rearrange("p (k m) -> p k m", k=MB),
                          we[:, :].rearrange("(k p) m -> p k m", p=128))
        fc1_sb = res.tile([128, MB * F], bf16)
        nc.sync.dma_start(fc1_sb[:].rearrange("p (k m) -> p k m", k=MB),
                          fc1T[:, :].rearrange("(k p) m -> p k m", p=128))
        acat_sb = res.tile([128, MB * 16], bf16)
        nc.sync.dma_start(acat_sb[:].rearrange("p (k m) -> p k m", k=MB),
                          acat[:, :].rearrange("(k p) m -> p k m", p=128))
        fc2_sb = res.tile([128, MB * OUT], bf16)
        nc.sync.dma_start(fc2_sb[:].rearrange("p (k m) -> p k m", k=MB),
                          fc2T[:, :].rearrange("(k p) m -> p k m", p=128))
        a2s_sb = res.tile([128, OUT], f32)
        nc.sync.dma_start(a2s_sb[:], a2s[:, :])
        a2d_sb = res.tile([128, OUT], f32)
        nc.sync.dma_start(a2d_sb[:], a2d[:, :])
        bv_sb = res.tile([128, MB], f32)
        nc.sync.dma_start(bv_sb[:], biasv[:, :])
        be_sb = res.tile([128, MB], f32)
        nc.sync.dma_start(be_sb[:], biase[:, :])
        iota_sb = res.tile([128, 128], f32)
        nc.sync.dma_start(iota_sb[:], iota[:, :])
        id_sb = res.tile([128, 128], bf16)
        nc.sync.dma_start(id_sb[:], ident[:, :])
        g1_sb = res.tile([128, S1 // 16], i16)
        nc.sync.dma_start(g1_sb[:], g1[:, :])
        sd1_sb = res.tile([128, S1 // 16], i16)
        nc.sync.dma_start(sd1_sb[:], sd1[:, :])
        dc1_sb = res.tile([128, S1 // 128], f32)
        nc.sync.dma_start(dc1_sb[:], dc1[:, :])
        g2_sb = res.tile([128, S2 // 16], i16)
        nc.sync.dma_start(g2_sb[:], g2[:, :])
        sd2_sb = res.tile([128, S2 // 16], i16)
        nc.sync.dma_start(sd2_sb[:], sd2[:, :])
        dc2_sb = res.tile([128, S2 // 128], f32)
        nc.sync.dma_start(dc2_sb[:], dc2[:, :])

        # chain for timing
        tk = res.tile([128, 1], f32)
        nc.sync.dma_start(tk[:], tick[:, :])
        nc.sync.dma_start(tock[:, :], tk[:])
        if stop < 9:
            zo = res.tile([128, OUT], f32)
            nc.vector.memset(zo[:], 0.0)
            for tt in range(_ceil(DS1, 128)):
                rows = min(128, DS1 - tt * 128)
                nc.sync.dma_start(out[tt * 128:tt * 128 + rows, :],
                                  zo[:rows, :])

        for _rep in range(REPEAT):
          if _rep:
              # serialize repeats so repeat-K timing measures a full
              # dependency-honest iteration (idempotent reps would
              # otherwise overlap through untracked DRAM reuse)
              tc.strict_bb_all_engine_barrier()
          # ---------------- Phase A ----------------
          WA = min(500, SS)
          with ExitStack() as pa:
              rhsp = pa.enter_context(tc.tile_pool(name=f"parhs{_rep}", bufs=2))
              sbp = pa.enter_context(tc.tile_pool(name=f"pasb{_rep}", bufs=2))
              psp = pa.enter_context(tc.tile_pool(name=f"paps{_rep}", bufs=4, space="PSUM"))
              pst = pa.enter_context(tc.tile_pool(name=f"patr{_rep}", bufs=2, space="PSUM"))
              stp = pa.enter_context(tc.tile_pool(name=f"past{_rep}", bufs=3))
              for nt in range(_ceil(SS, WA)):
                  n0 = nt * WA
                  w = min(WA, SS - n0)
                  x_sb = rhsp.tile([128, KI * w], bf16, tag="x")
                  nc.sync.dma_start(
                      x_sb[:].rearrange("p (k n) -> p k n", k=KI),
                      imgT[:, n0:n0 + w].rearrange("(k p) n -> p k n", p=128))
                  b_sb = rhsp.tile([128, KB * w], bf16, tag="b")
                  nc.sync.dma_start(
                      b_sb[:].rearrange("p (k n) -> p k n", k=KB),
                      blkT[:, n0:n0 + w].rearrange("(k p) n -> p k n", p=128))

                  def mm(lhs_sb, rhs_sb, K, m, width):
                      ps = psp.tile([128, width], f32, tag="ps")
                      for k in range(K):
                          nc.tensor.matmul(
                              ps[:],
                              lhs_sb[:, (k * F + m * 128):(k * F + m * 128) + 128],
                              rhs_sb[:, k * width:(k + 1) * width],
                              start=(k == 0), stop=(k == K - 1))
                      return ps

                  fi_sb = sbp.tile([128, MB * w], bf16, tag="fi")
                  ti_sb = sbp.tile([128, MB * w], bf16, tag="ti")
                  av_sb = sbp.tile([128, MB * w], bf16, tag="av")
                  ae_sb = sbp.tile([128, MB * w], bf16, tag="ae")
                  for m in range(MB):
                      ps = mm(wimg_sb, x_sb, KI, m, w)
                      nc.vector.tensor_copy(fi_sb[:, m * w:(m + 1) * w], ps[:])
                  for m in range(MB):
                      ps = mm(wblk_sb, b_sb, KB, m, w)
                      nc.vector.tensor_copy(ti_sb[:, m * w:(m + 1) * w], ps[:])
                  for m in range(MB):
                      ps = mm(wv_sb, fi_sb, MB, m, w)
                      nc.scalar.activation(av_sb[:, m * w:(m + 1) * w], ps[:],
                                           Sig, bias=bv_sb[:, m:m + 1])
                  for m in range(MB):
                      ps = mm(we_sb, ti_sb, MB, m, w)
                      nc.scalar.activation(ae_sb[:, m * w:(m + 1) * w], ps[:],
                                           Sig, bias=be_sb[:, m:m + 1])
                  fu_sb = sbp.tile([128, MB * w], bf16, tag="fu")
                  TT(fu_sb[:], av_sb[:], fi_sb[:], MUL)
                  TT(ae_sb[:], ae_sb[:], ti_sb[:], MUL)
                  TT(fu_sb[:], fu_sb[:], ae_sb[:], ADD)
                  z_sb = sbp.tile([128, MB * w], bf16, tag="z")
                  for m in range(MB):
                      ps = mm(fc1_sb, fu_sb, MB, m, w)
                      nc.vector.tensor_copy(z_sb[:, m * w:(m + 1) * w], ps[:])
                  pss = psp.tile([128, w], f32, tag="ps")
                  for k in range(MB):
                      nc.tensor.matmul(pss[:16, :], acat_sb[:, k * 16:(k + 1) * 16],
                                       fu_sb[:, k * w:(k + 1) * w],
                                       start=(k == 0), stop=(k == MB - 1))
                  s_sb = sbp.tile([16, w], bf16, tag="s")
                  nc.vector.tensor_copy(s_sb[:], pss[:16, :])
                  for b0 in range(0, w, 128):
                      wb = min(128, w - b0)
                      st = stp.tile([128, ROW1], bf16, tag="t1")
                      nc.vector.memset(st[:, F + 16:ROW1], 0.0)
                      for m in range(MB):
                          ptr = pst.tile([128, 128], bf16, tag="tr")
                          nc.tensor.matmul(ptr[:wb, :],
                                           z_sb[:, m * w + b0:m * w + b0 + wb],
                                           id_sb[:], is_transpose=True)
                          nc.vector.tensor_copy(
                              st[:wb, m * 128:(m + 1) * 128], ptr[:wb, :])
                      ptr = pst.tile([128, 128], bf16, tag="tr")
                      nc.tensor.matmul(ptr[:wb, :16], s_sb[:, b0:b0 + wb],
                                       id_sb[:16, :16], is_transpose=True)
                      nc.vector.tensor_copy(st[:wb, F:F + 16], ptr[:wb, :16])
                      nc.sync.dma_start(table1[n0 + b0:n0 + b0 + wb, :],
                                        st[:wb, :])
                      st2 = stp.tile([128, SROW], bf16, tag="t2")
                      nc.vector.memset(st2[:, 8:SROW], 0.0)
                      nc.vector.tensor_copy(st2[:wb, 0:8], st[:wb, F + 8:F + 16])
                      nc.sync.dma_start(sdsh1[n0 + b0:n0 + b0 + wb, :],
                                        st2[:wb, :])

          if stop >= 2:
              nc.gpsimd.collective_compute(
                  "AllGather", mybir.AluOpType.bypass,
                  replica_groups=[list(range(NCORE))],
                  ins=[sdsh1[:, :]], outs=[sdt1[:, :]])

          # ---------------- Layer-1 aggregation ----------------
          def _rs1a():
              nc.gpsimd.collective_compute(
                  "ReduceScatter", ADD, replica_groups=[list(range(NCORE))],
                  ins=[partials1a[:, :]], outs=[rs1a[:, :]])

          with ExitStack() as ag1:
            if stop >= 3:
              _agg_layer(nc, tc, ag1, cfg, table=table1, sdt=sdt1,
                         gsb=g1_sb, sdsb=sd1_sb, dcsb=dc1_sb, iota_sb=iota_sb,
                         calls=cfg["calls1"], cm=cfg["cm1"],
                         row=ROW1, srow=SROW, prow=PR1, zw=F, nh=H,
                         pa=partials1a, pb=partials1b, routing=cfg["rt1"],
                         hooks=({cfg["sp1"]: _rs1a} if stop >= 4 else None),
                         rep=_rep)

          if stop >= 4:
              nc.gpsimd.collective_compute(
                  "ReduceScatter", ADD, replica_groups=[list(range(NCORE))],
                  ins=[partials1b[:, :]], outs=[rs1b[:, :]])

          # ---------------- normalize + layer-2 tables ----------------
          with ExitStack() as p4:
            if stop >= 5:
              sbp = p4.enter_context(tc.tile_pool(name=f"n2sb{_rep}", bufs=3))
              psp = p4.enter_context(tc.tile_pool(name=f"n2ps{_rep}", bufs=2, space="PSUM"))
              ptp = p4.enter_context(tc.tile_pool(name=f"n2pt{_rep}", bufs=2, space="PSUM"))
              for tt in range(_ceil(DS0, 128)):
                  r0 = tt * 128
                  rows = min(128, DS0 - r0)
                  hs = sbp.tile([128, PR1], bf16, tag="hs")
                  for (h_sel, h_lo, h_ln, h_dst) in _split_rows(
                          r0, rows, half1):
                      nc.sync.dma_start(
                          hs[h_lo:h_lo + h_ln, :],
                          (rs1a if h_sel == 0 else rs1b)[
                              h_dst:h_dst + h_ln, :])
                  rden = sbp.tile([128, H], f32, tag="rd")
                  nc.vector.reciprocal(rden[:rows, :], hs[:rows, F:F + H])
                  hraw = sbp.tile([128, F], f32, tag="hraw")
                  TT(hraw[:rows, :].rearrange("p (h o) -> p h o", h=H),
                     hs[:rows, 0:F].rearrange("p (h o) -> p h o", h=H),
                     rden[:rows, :].unsqueeze(2).broadcast_to(
                         [rows, H, F // H]),
                     MUL)
                  t1 = sbp.tile([128, F], f32, tag="t1")
                  nc.vector.tensor_scalar_min(t1[:rows, :], hraw[:rows, :], 0.0)
                  nc.scalar.activation(t1[:rows, :], t1[:rows, :], Exp)
                  h1 = sbp.tile([128, F], bf16, tag="h1")
                  nc.vector.scalar_tensor_tensor(
                      h1[:rows, :], t1[:rows, :], -1.0, hraw[:rows, :],
                      ADD, mybir.AluOpType.max)
                  h1t = sbp.tile([128, MB * 128], bf16, tag="h1t")
                  for m in range(MB):
                      ptr = ptp.tile([128, 128], bf16, tag="tr")
                      nc.tensor.matmul(ptr[:, :rows],
                                       h1[:rows, m * 128:(m + 1) * 128],
                                       id_sb[:rows, :rows], is_transpose=True)
                      nc.vector.tensor_copy(h1t[:, m * 128:m * 128 + rows],
                                            ptr[:, :rows])
                  pz2 = psp.tile([128, OUT], f32, tag="z2")
                  for k in range(MB):
                      nc.tensor.matmul(pz2[:rows, :],
                                       h1t[:, k * 128:k * 128 + rows],
                                       fc2_sb[:, k * OUT:(k + 1) * OUT],
                                       start=(k == 0), stop=(k == MB - 1))
                  scr = sbp.tile([128, OUT], f32, tag="scr")
                  s2s = sbp.tile([128, 1], f32, tag="s2s")
                  s2d = sbp.tile([128, 1], f32, tag="s2d")
                  if P4_LEVEL >= 3:
                      nc.vector.tensor_tensor_reduce(
                          scr[:rows, :], pz2[:rows, :], a2s_sb[:rows, :], 1.0,
                          0.0, MUL, ADD, s2s[:rows, :])
                      nc.vector.tensor_tensor_reduce(
                          scr[:rows, :], pz2[:rows, :], a2d_sb[:rows, :], 1.0,
                          0.0, MUL, ADD, s2d[:rows, :])
                  else:
                      nc.vector.tensor_tensor(scr[:rows, :], pz2[:rows, :],
                                              a2s_sb[:rows, :], MUL)
                      nc.vector.reduce_sum(s2s[:rows, :], scr[:rows, :],
                                           mybir.AxisListType.X)
                      nc.vector.tensor_tensor(scr[:rows, :], pz2[:rows, :],
                                              a2d_sb[:rows, :], MUL)
                      nc.vector.reduce_sum(s2d[:rows, :], scr[:rows, :],
                                           mybir.AxisListType.X)
                  st = sbp.tile([128, ROW2], bf16, tag="st")
                  nc.vector.memset(st[:, OUT + 1:ROW2], 0.0)
                  nc.vector.tensor_copy(st[:rows, 0:OUT], pz2[:rows, :])
                  nc.vector.tensor_copy(st[:rows, OUT:OUT + 1], s2s[:rows, :])
                  nc.sync.dma_start(table2[r0:r0 + rows, :], st[:rows, :])
                  st2 = sbp.tile([128, SROW], bf16, tag="st2")
                  nc.vector.memset(st2[:, 1:SROW], 0.0)
                  nc.vector.tensor_copy(st2[:rows, 0:1], s2d[:rows, :])
                  nc.sync.dma_start(sdsh2[r0:r0 + rows, :], st2[:rows, :])

          if stop >= 6:
              nc.gpsimd.collective_compute(
                  "AllGather", mybir.AluOpType.bypass,
                  replica_groups=[list(range(NCORE))],
                  ins=[sdsh2[:, :]], outs=[sdt2[:, :]])

          # ---------------- Layer-2 aggregation ----------------
          def _rs2a():
              nc.gpsimd.collective_compute(
                  "ReduceScatter", ADD, replica_groups=[list(range(NCORE))],
                  ins=[partials2a[:, :]], outs=[rs2a[:, :]])

          with ExitStack() as ag2:
            if stop >= 7:
              _agg_layer(nc, tc, ag2, cfg, table=table2, sdt=sdt2,
                         gsb=g2_sb, sdsb=sd2_sb, dcsb=dc2_sb, iota_sb=iota_sb,
                         calls=cfg["calls2"], cm=cfg["cm2"],
                         row=ROW2, srow=SROW, prow=PR2, zw=OUT, nh=1,
                         pa=partials2a, pb=partials2b, routing=cfg["rt2"],
                         hooks=({cfg["sp2"]: _rs2a} if stop >= 8 else None),
                         rep=_rep)

          if stop >= 8:
              nc.gpsimd.collective_compute(
                  "ReduceScatter", ADD, replica_groups=[list(range(NCORE))],
                  ins=[partials2b[:, :]], outs=[rs2b[:, :]])

          # ---------------- final normalize ----------------
          with ExitStack() as p8:
            if stop >= 9:
              sbp = p8.enter_context(tc.tile_pool(name=f"fsb{_rep}", bufs=3))
              for tt in range(_ceil(DS1, 128)):
                  r0 = tt * 128
                  rows = min(128, DS1 - r0)
                  hs = sbp.tile([128, PR2], bf16, tag="hs")
                  for (h_sel, h_lo, h_ln, h_dst) in _split_rows(
                          r0, rows, half2):
                      nc.sync.dma_start(
                          hs[h_lo:h_lo + h_ln, :],
                          (rs2a if h_sel == 0 else rs2b)[
                              h_dst:h_dst + h_ln, :])
                  rden = sbp.tile([128, 1], f32, tag="rd")
                  nc.vector.reciprocal(rden[:rows, :], hs[:rows, OUT:OUT + 1])
                  ot = sbp.tile([128, OUT], f32, tag="ot")
                  TT(ot[:rows, :], hs[:rows, 0:OUT],
                     rden[:rows, :].broadcast_to([rows, OUT]), MUL)
                  nc.sync.dma_start(out[r0:r0 + rows, :], ot[:rows, :])

    nc.compile()
    return nc


_CACHE = {}


def _get_nc(cfg):
    key = repr(sorted((k, v) for k, v in cfg.items()))
    if key not in _CACHE:
        _CACHE[key] = _build(cfg)
    return _CACHE[key]


def kernel(**inputs) -> np.ndarray:
    cfg, in_maps = _prep(inputs)
    nc = _get_nc(cfg)
    res = run_bass_kernel_spmd(nc, in_maps, core_ids=list(range(NCORE)))
    return np.concatenate([res.results[c]["out"] for c in range(NCORE)],
                          axis=0)

